# revision 1
# baseline (speedup 1.0000x reference)
"""CrossAttention (cosine-normalized QK) Trainium2 Bass kernel, 8-core SPMD.

Sharding: batch (2) x query-row blocks (4) -> 8 cores. Each core computes the
full K/V projection for its batch (replicated within a batch group) and a
512-row slice of queries; output rows are disjoint, so the gather is a pure
concatenation (no collectives).

v2: K-projection is interleaved with attention in 512-key blocks so the
PE-heavy projection overlaps the ACT-heavy softmax exp; attention partials
accumulate in SBUF fp32.
"""

import numpy as np
import ml_dtypes
from contextlib import ExitStack

import concourse.bacc as bacc
import concourse.bass as bass
import concourse.mybir as mybir
import concourse.tile as tile
from concourse import bass_utils

F32 = mybir.dt.float32
BF16 = mybir.dt.bfloat16
AF = mybir.ActivationFunctionType

B, NQ, NK = 2, 2048, 2048
QD, KD, E, H = 1024, 768, 1024, 16
D = E // H          # 64
NC = 8              # cores
NQC = NQ * B // NC  # 512 query rows per core
SCALE = D ** -0.5   # 0.125
LN_EPS = 1e-5

IC_Q = QD // 128    # 8  contraction chunks for Q proj
IC_K = KD // 128    # 6  contraction chunks for K/V proj
EC = E // 128       # 8  embed chunks
KC = NK // 128      # 16 key chunks
NT = NQC // 128     # 4  query-row tiles
HP = H // 2         # 8  head pairs
KS = 4              # key super-blocks (512 keys each)


def build():
    nc = bacc.Bacc("TRN2", target_bir_lowering=False, debug=False,
                   enable_asserts=False, num_devices=1)

    qT = nc.dram_tensor("qT", [QD, NQC], BF16, kind="ExternalInput").ap()
    kT = nc.dram_tensor("kT", [KD, NK], BF16, kind="ExternalInput").ap()
    vT = nc.dram_tensor("vT", [KD, NK], BF16, kind="ExternalInput").ap()
    wq = nc.dram_tensor("wq", [QD, E], BF16, kind="ExternalInput").ap()
    wk = nc.dram_tensor("wk", [KD, E], BF16, kind="ExternalInput").ap()
    wv = nc.dram_tensor("wv", [KD, E], BF16, kind="ExternalInput").ap()
    wo = nc.dram_tensor("wo", [E, E], BF16, kind="ExternalInput").ap()
    bq = nc.dram_tensor("bq", [E], F32, kind="ExternalInput").ap()
    bk_pp = nc.dram_tensor("bk_pp", [128, EC], F32, kind="ExternalInput").ap()
    bv = nc.dram_tensor("bv", [E], F32, kind="ExternalInput").ap()
    bo = nc.dram_tensor("bo", [E], F32, kind="ExternalInput").ap()
    gam = nc.dram_tensor("gam", [E], F32, kind="ExternalInput").ap()
    bet = nc.dram_tensor("bet", [E], F32, kind="ExternalInput").ap()
    out = nc.dram_tensor("out", [NQC, E], F32, kind="ExternalOutput").ap()

    def bcast_row(vec_ap, parts=128):
        return bass.AP(tensor=vec_ap.tensor, offset=vec_ap.offset,
                       ap=[[0, parts], [1, vec_ap.shape[0]]])

    with tile.TileContext(nc) as tc, ExitStack() as ctx:
        # ---- persistent pools -------------------------------------------
        per = ctx.enter_context(tc.tile_pool(name="per", bufs=1))
        dram = ctx.enter_context(tc.tile_pool(name="dram", bufs=1, space="DRAM"))

        v_sb = per.tile([128, KC, H, D + 1], BF16)      # V with ones col
        kpT_sb = per.tile([128, EC, NK], BF16)          # K proj, transposed
        qnT_sb = per.tile([128, EC, NQC], BF16)         # normalized Q, transposed
        aoT_sb = per.tile([128, EC, NQC], BF16)         # attn out, transposed
        rk_pp = per.tile([128, KC], F32)                # 0.125/||k|| per key
        rk_bf = per.tile([128, KC], BF16)
        ones128 = per.tile([128, 1], BF16)
        nc.vector.memset(ones128, 1.0)
        nc.vector.memset(v_sb[:, :, :, D:D + 1], 1.0)
        eps24 = per.tile([128, 1], F32)
        nc.vector.memset(eps24, 1e-24)
        epsln = per.tile([128, 1], F32)
        nc.vector.memset(epsln, LN_EPS)
        bk_sb = per.tile([128, EC], F32)
        nc.sync.dma_start(out=bk_sb, in_=bk_pp)

        qn_dram = dram.tile([NQC, E], BF16)
        qp_dram = dram.tile([NQC, E], F32)
        rk_dram = dram.tile([1, NK], BF16)

        # ---- phase A: V = value @ Wv + bv  (natural, +ones col) ---------
        with tc.tile_pool(name="pa", bufs=1) as pa, \
             tc.tile_pool(name="psv", bufs=4, space="PSUM") as psv:
            vT_sb = pa.tile([128, IC_K, NK], BF16)
            wv_sb = pa.tile([128, IC_K, E], BF16)
            bv_bc = pa.tile([128, E], F32)
            nc.sync.dma_start(out=vT_sb, in_=vT.rearrange("(c p) n -> p c n", p=128))
            nc.sync.dma_start(out=wv_sb, in_=wv.rearrange("(c p) e -> p c e", p=128))
            nc.gpsimd.dma_start(out=bv_bc, in_=bcast_row(bv))
            for kc in range(KC):
                for ec in range(2):
                    ps_v = psv.tile([128, 512], F32)
                    for ic in range(IC_K):
                        nc.tensor.matmul(ps_v,
                                         vT_sb[:, ic, kc * 128:(kc + 1) * 128],
                                         wv_sb[:, ic, ec * 512:(ec + 1) * 512],
                                         start=(ic == 0), stop=(ic == IC_K - 1))
                    nc.vector.tensor_add(
                        out=v_sb[:, kc, ec * 8:(ec + 1) * 8, 0:D],
                        in0=ps_v.rearrange("p (h d) -> p h d", d=D),
                        in1=bv_bc[:, ec * 512:(ec + 1) * 512].rearrange(
                            "p (h d) -> p h d", d=D))

        # ---- phase C: Qp natural + residual(->DRAM) + Qn^T --------------
        with tc.tile_pool(name="pc", bufs=1) as pc, \
             tc.tile_pool(name="psq", bufs=2, space="PSUM") as psq, \
             tc.tile_pool(name="qsc", bufs=2) as qsc:
            qT_sb = pc.tile([128, IC_Q, NQC], BF16)
            wq_sb = pc.tile([128, IC_Q, E], BF16)
            bq_bc = pc.tile([128, E], F32)
            nc.sync.dma_start(out=qT_sb, in_=qT.rearrange("(c p) n -> p c n", p=128))
            nc.sync.dma_start(out=wq_sb, in_=wq.rearrange("(c p) e -> p c e", p=128))
            nc.gpsimd.dma_start(out=bq_bc, in_=bcast_row(bq))
            for nt in range(NT):
                ps_q = psq.tile([128, E], F32)
                for half in range(2):
                    for ic in range(IC_Q):
                        nc.tensor.matmul(ps_q[:, half * 512:(half + 1) * 512],
                                         qT_sb[:, ic, nt * 128:(nt + 1) * 128],
                                         wq_sb[:, ic, half * 512:(half + 1) * 512],
                                         start=(ic == 0), stop=(ic == IC_Q - 1))
                qp_st = qsc.tile([128, E], F32, tag="qpst")
                nc.vector.tensor_add(out=qp_st, in0=ps_q, in1=bq_bc)
                nc.sync.dma_start(out=qp_dram[nt * 128:(nt + 1) * 128, :], in_=qp_st)
                sq_q = qsc.tile([128, E], F32, tag="sqq")
                nc.vector.tensor_mul(out=sq_q, in0=qp_st, in1=qp_st)
                ssq = qsc.tile([128, 1], F32, tag="ssq")
                nc.vector.reduce_sum(out=ssq, in_=sq_q, axis=mybir.AxisListType.X)
                nc.scalar.activation(out=ssq, in_=ssq, func=AF.Sqrt,
                                     bias=eps24, scale=1.0)
                rq_t = qsc.tile([128, 1], F32, tag="rqt")
                nc.vector.reciprocal(out=rq_t, in_=ssq)
                qn_st = qsc.tile([128, E], BF16, tag="qnst")
                nc.scalar.activation(out=qn_st, in_=qp_st,
                                     func=AF.Identity, scale=rq_t, bias=0.0)
                nc.sync.dma_start(out=qn_dram[nt * 128:(nt + 1) * 128, :], in_=qn_st)
            for ec in range(EC):
                nc.sync.dma_start(out=qnT_sb[:, ec, :],
                                  in_=qn_dram[:, ec * 128:(ec + 1) * 128],
                                  transpose=True)

        # ---- interleaved: K-proj block ks  +  attention over block ks ---
        with tc.tile_pool(name="pb", bufs=1) as pb, \
             tc.tile_pool(name="acp", bufs=1) as acp, \
             tc.tile_pool(name="sqp", bufs=3) as sqp, \
             tc.tile_pool(name="esp", bufs=3) as esp, \
             tc.tile_pool(name="psk", bufs=2, space="PSUM") as psk, \
             tc.tile_pool(name="pss", bufs=1, space="PSUM") as pss, \
             tc.tile_pool(name="ps_s", bufs=1, space="PSUM") as ps_sp, \
             tc.tile_pool(name="ps_o", bufs=2, space="PSUM") as ps_op:
            kT_sb = pb.tile([128, IC_K, NK], BF16)
            wk_sb = pb.tile([128, IC_K, E], BF16)
            nc.sync.dma_start(out=kT_sb, in_=kT.rearrange("(c p) n -> p c n", p=128))
            nc.sync.dma_start(out=wk_sb, in_=wk.rearrange("(c p) e -> p c e", p=128))
            acc = acp.tile([128, H, NQC], F32)   # rows 0..63 outT, row 64 rowsum

            for ks in range(KS):
                # -- K proj for keys [ks*512, (ks+1)*512) --
                ps_ss = pss.tile([1, 512], F32)
                for ec in range(EC):
                    ps_k = psk.tile([128, 512], F32)
                    for ic in range(IC_K):
                        nc.tensor.matmul(ps_k,
                                         wk_sb[:, ic, ec * 128:(ec + 1) * 128],
                                         kT_sb[:, ic, ks * 512:(ks + 1) * 512],
                                         start=(ic == 0), stop=(ic == IC_K - 1))
                    kslice = kpT_sb[:, ec, ks * 512:(ks + 1) * 512]
                    nc.vector.tensor_scalar_add(out=kslice, in0=ps_k,
                                                scalar1=bk_sb[:, ec:ec + 1])
                    sq = sqp.tile([128, 512], BF16)
                    nc.vector.tensor_mul(out=sq, in0=kslice, in1=kslice)
                    nc.tensor.matmul(ps_ss, ones128, sq,
                                     start=(ec == 0), stop=(ec == EC - 1))
                srt = sqp.tile([1, 512], F32, tag="srt")
                nc.scalar.activation(out=srt, in_=ps_ss, func=AF.Sqrt,
                                     bias=eps24[0:1, :], scale=1.0)
                rec = sqp.tile([1, 512], F32, tag="rec")
                nc.vector.reciprocal(out=rec, in_=srt)
                rkb = sqp.tile([1, 512], BF16, tag="rkb")
                nc.scalar.mul(out=rkb, in_=rec, mul=SCALE)
                nc.sync.dma_start(out=rk_dram[:, ks * 512:(ks + 1) * 512], in_=rkb)
                nc.sync.dma_start(
                    out=rk_bf[:, ks * 4:(ks + 1) * 4],
                    in_=rk_dram[:, ks * 512:(ks + 1) * 512].rearrange(
                        "one (a b) -> b (one a)", b=128))
                nc.vector.tensor_copy(out=rk_pp[:, ks * 4:(ks + 1) * 4],
                                      in_=rk_bf[:, ks * 4:(ks + 1) * 4])

                # -- attention over this key block, all head pairs --
                for hp in range(HP):
                    po = [ps_op.tile([D + 1, NQC], F32, tag="po",
                                     name=f"po{ks}_{hp}_{j}") for j in range(2)]
                    for j in range(4):
                        kc = ks * 4 + j
                        ps_s = ps_sp.tile([128, 2 * NQC], F32)
                        for i in range(2):
                            nc.tensor.matmul(
                                ps_s[:, i * NQC:(i + 1) * NQC],
                                kpT_sb[i * D:(i + 1) * D, hp,
                                       kc * 128:(kc + 1) * 128],
                                qnT_sb[i * D:(i + 1) * D, hp, :],
                                start=True, stop=True)
                        es = esp.tile([128, 2 * NQC], BF16)
                        nc.scalar.activation(out=es, in_=ps_s, func=AF.Exp,
                                             scale=rk_pp[:, kc:kc + 1], bias=0.0)
                        for i in range(2):
                            nc.tensor.matmul(po[i],
                                             v_sb[:, kc, 2 * hp + i, :],
                                             es[:, i * NQC:(i + 1) * NQC],
                                             start=(j == 0), stop=(j == 3))
                    for i in range(2):
                        h = 2 * hp + i
                        if ks == 0:
                            nc.vector.tensor_copy(out=acc[0:D + 1, h, :],
                                                  in_=po[i])
                        else:
                            nc.vector.tensor_add(out=acc[0:D + 1, h, :],
                                                 in0=acc[0:D + 1, h, :],
                                                 in1=po[i])

            # -- normalize: aoT = acc / rowsum ----------------------------
            with tc.tile_pool(name="nrm", bufs=4) as nrm, \
                 tc.tile_pool(name="drm", bufs=4, space="DRAM") as drm:
                for h in range(H):
                    rec2 = nrm.tile([1, NQC], F32, tag="rec2")
                    nc.vector.reciprocal(out=rec2, in_=acc[D:D + 1, h, :])
                    rdr = drm.tile([1, NQC], F32)
                    nc.sync.dma_start(out=rdr, in_=rec2)
                    rbc = nrm.tile([D, NQC], F32, tag="rbc")
                    nc.sync.dma_start(
                        out=rbc, in_=bass.AP(tensor=rdr.tensor, offset=rdr.offset,
                                             ap=[[0, D], [1, NQC]]))
                    nc.vector.tensor_mul(
                        out=aoT_sb[(h % 2) * D:(h % 2 + 1) * D, h // 2, :],
                        in0=acc[0:D, h, :], in1=rbc)

        # ---- phase E: out proj + residual + layernorm -------------------
        with tc.tile_pool(name="pe", bufs=1) as pe, \
             tc.tile_pool(name="lnp", bufs=2) as lnp, \
             tc.tile_pool(name="psf", bufs=2, space="PSUM") as psf:
            wo_sb = pe.tile([128, EC, E], BF16)
            bo_bc = pe.tile([128, E], F32)
            gam_bc = pe.tile([128, E], F32)
            bet_bc = pe.tile([128, E], F32)
            nc.sync.dma_start(out=wo_sb, in_=wo.rearrange("(c p) e -> p c e", p=128))
            nc.gpsimd.dma_start(out=bo_bc, in_=bcast_row(bo))
            nc.gpsimd.dma_start(out=gam_bc, in_=bcast_row(gam))
            nc.gpsimd.dma_start(out=bet_bc, in_=bcast_row(bet))
            for nt in range(NT):
                ps_f = psf.tile([128, E], F32)
                for half in range(2):
                    for fc in range(EC):
                        nc.tensor.matmul(ps_f[:, half * 512:(half + 1) * 512],
                                         aoT_sb[:, fc, nt * 128:(nt + 1) * 128],
                                         wo_sb[:, fc, half * 512:(half + 1) * 512],
                                         start=(fc == 0), stop=(fc == EC - 1))
                qp_ld = lnp.tile([128, E], F32, tag="qpld")
                nc.sync.dma_start(out=qp_ld,
                                  in_=qp_dram[nt * 128:(nt + 1) * 128, :])
                xs = lnp.tile([128, E], F32, tag="xs")
                nc.vector.tensor_add(out=xs, in0=ps_f, in1=bo_bc)
                nc.vector.tensor_add(out=xs, in0=xs, in1=qp_ld)
                stats = lnp.tile([128, 2, 6], F32, tag="st")
                xs3 = xs.rearrange("p (a b) -> p a b", b=512)
                for sg in range(2):
                    nc.vector.bn_stats(out=stats[:, sg, :], in_=xs3[:, sg, :])
                mv = lnp.tile([128, 2], F32, tag="mv")
                nc.vector.bn_aggr(out=mv, in_=stats)
                rstd = lnp.tile([128, 1], F32, tag="rstd")
                nc.scalar.activation(out=rstd, in_=mv[:, 1:2], func=AF.Sqrt,
                                     bias=epsln, scale=1.0)
                nc.vector.reciprocal(out=rstd, in_=rstd)
                nmr = lnp.tile([128, 1], F32, tag="nmr")
                nc.vector.tensor_mul(out=nmr, in0=mv[:, 0:1], in1=rstd)
                nc.scalar.mul(out=nmr, in_=nmr, mul=-1.0)
                xn = lnp.tile([128, E], F32, tag="xn")
                nc.scalar.activation(out=xn, in_=xs, func=AF.Identity,
                                     scale=rstd, bias=nmr)
                nc.vector.tensor_mul(out=xn, in0=xn, in1=gam_bc)
                ot = lnp.tile([128, E], F32, tag="ot")
                nc.vector.tensor_add(out=ot, in0=xn, in1=bet_bc)
                nc.sync.dma_start(out=out[nt * 128:(nt + 1) * 128, :], in_=ot)

    nc.compile()
    return nc


_NC_CACHE = None
_last_in_maps = None


def _get_nc():
    global _NC_CACHE
    if _NC_CACHE is None:
        _NC_CACHE = build()
    return _NC_CACHE


def kernel(**inputs):
    q = np.asarray(inputs["query"], np.float32)
    k = np.asarray(inputs["key"], np.float32)
    v = np.asarray(inputs["value"], np.float32)
    Wq = np.asarray(inputs["Wq"], np.float32).astype(ml_dtypes.bfloat16)
    Wk = np.asarray(inputs["Wk"], np.float32).astype(ml_dtypes.bfloat16)
    Wv = np.asarray(inputs["Wv"], np.float32).astype(ml_dtypes.bfloat16)
    Wo = np.asarray(inputs["Wo"], np.float32).astype(ml_dtypes.bfloat16)
    bq = np.asarray(inputs["bq"], np.float32)
    bk = np.asarray(inputs["bk"], np.float32)
    bv = np.asarray(inputs["bv"], np.float32)
    bo = np.asarray(inputs["bo"], np.float32)
    gam = np.asarray(inputs["ln_gamma"], np.float32)
    bet = np.asarray(inputs["ln_beta"], np.float32)

    bk_pp = np.ascontiguousarray(bk.reshape(EC, 128).T)
    kTs = [np.ascontiguousarray(k[b].T.astype(ml_dtypes.bfloat16)) for b in range(B)]
    vTs = [np.ascontiguousarray(v[b].T.astype(ml_dtypes.bfloat16)) for b in range(B)]

    in_maps = []
    for c in range(NC):
        b, r0 = c // 4, (c % 4) * NQC
        qTa = np.ascontiguousarray(q[b, r0:r0 + NQC, :].T.astype(ml_dtypes.bfloat16))
        in_maps.append({
            "qT": qTa, "kT": kTs[b], "vT": vTs[b],
            "wq": Wq, "wk": Wk, "wv": Wv, "wo": Wo,
            "bq": bq, "bk_pp": bk_pp, "bv": bv, "bo": bo,
            "gam": gam, "bet": bet,
        })

    global _last_in_maps
    _last_in_maps = in_maps
    nc = _get_nc()
    res = bass_utils.run_bass_kernel_spmd(nc, in_maps, core_ids=list(range(NC)))

    out = np.empty((B, NQ, E), np.float32)
    for c in range(NC):
        b, r0 = c // 4, (c % 4) * NQC
        out[b, r0:r0 + NQC, :] = res.results[c]["out"]
    return out



# revision 25
# speedup vs baseline: 1.8571x; 1.8571x over previous
"""CrossAttention (cosine-normalized QK) Trainium2 Bass kernel, 8-core SPMD.

Sharding: batch (2) x query-row blocks (4) -> 8 cores. Each core computes the
full K/V projection for its batch (replicated within a batch group) and a
512-row slice of queries; output rows are disjoint, so the gather is a pure
concatenation (no collectives).

v4 (~320us, vs ~600us v2 baseline): engine-overlap structure.
 - Phases: K proj -> Q proj (+16 V-proj chains riding Q's idle PE) ->
   attention (+16 more V chains riding the exp-bound PE) -> O proj + LN.
 - Attention heads-outer / key-chunks-inner: PV accumulates all 16 key
   chunks in PSUM; QK emitted two chunks ahead of exp so PE stalls never
   starve ScalarE; exp stream runs ScalarE at ~100% (147us floor).
 - po is single-buffered; it is evacuated to SBUF with one DVE copy so the
   bank frees fast; softmax denominators via reciprocal_approx_fast (DVE)
   + gpsimd partition_broadcast (no DRAM roundtrip).
 - Biases folded into matmuls (ones-row trick) or ACT bias operand; PSUM->
   SBUF moves on whichever of ScalarE/DVE is idle in that phase; qnT via
   PE transpose; Qp residual kept in SBUF; input loads striped across both
   HWDGE queues with K's inputs first; O proj PSUM reuses the score pool
   (avoids a WAR stall + HAM cold-clock penalty at the tail).
"""

import numpy as np
import ml_dtypes
from contextlib import ExitStack

import concourse.bacc as bacc
import concourse.bass as bass
import concourse.mybir as mybir
import concourse.tile as tile
from concourse import bass_utils
from concourse.masks import make_identity

F32 = mybir.dt.float32
BF16 = mybir.dt.bfloat16
AF = mybir.ActivationFunctionType

B, NQ, NK = 2, 2048, 2048
QD, KD, E, H = 1024, 768, 1024, 16
D = E // H          # 64
NC = 8              # cores
NQC = NQ * B // NC  # 512 query rows per core
SCALE = D ** -0.5   # 0.125
LN_EPS = 1e-5

IC_Q = QD // 128    # 8  contraction chunks for Q proj
IC_K = KD // 128    # 6  contraction chunks for K/V proj
EC = E // 128       # 8  embed chunks
KC = NK // 128      # 16 key chunks
NT = NQC // 128     # 4  query-row tiles
HP = H // 2         # 8  head pairs


def build(biases_zero=False, ln_trivial=False):
    nc = bacc.Bacc("TRN2", target_bir_lowering=False, debug=False,
                   enable_asserts=False, num_devices=1)

    qT = nc.dram_tensor("qT", [QD, NQC], BF16, kind="ExternalInput").ap()
    kT = nc.dram_tensor("kT", [KD, NK], BF16, kind="ExternalInput").ap()
    vT = nc.dram_tensor("vT", [KD, NK], BF16, kind="ExternalInput").ap()
    wq = nc.dram_tensor("wq", [QD, E], BF16, kind="ExternalInput").ap()
    wk = nc.dram_tensor("wk", [KD, E], BF16, kind="ExternalInput").ap()
    wv = nc.dram_tensor("wv", [KD, E], BF16, kind="ExternalInput").ap()
    wo = nc.dram_tensor("wo", [E, E], BF16, kind="ExternalInput").ap()
    bq_r = nc.dram_tensor("bq_r", [1, E], BF16, kind="ExternalInput").ap()
    bv_r = nc.dram_tensor("bv_r", [1, E], BF16, kind="ExternalInput").ap()
    bo_r = nc.dram_tensor("bo_r", [1, E], BF16, kind="ExternalInput").ap()
    bk_pp = nc.dram_tensor("bk_pp", [128, EC], F32, kind="ExternalInput").ap()
    gam = nc.dram_tensor("gam", [E], F32, kind="ExternalInput").ap()
    bet = nc.dram_tensor("bet", [E], F32, kind="ExternalInput").ap()
    out = nc.dram_tensor("out", [NQC, E], F32, kind="ExternalOutput").ap()

    def bcast_rows(src_ap, parts, n):
        return bass.AP(tensor=src_ap.tensor, offset=src_ap.offset,
                       ap=[[0, parts], [1, n]])

    with tile.TileContext(nc) as tc, ExitStack() as ctx:
        # ---- persistent tiles -------------------------------------------
        per = ctx.enter_context(tc.tile_pool(name="per", bufs=1))
        dram = ctx.enter_context(tc.tile_pool(name="dram", bufs=1, space="DRAM"))

        kpT_sb = per.tile([128, EC, NK], BF16)          # K proj, transposed
        v_sb = per.tile([128, KC, H, D + 1], BF16)      # V + ones col per head
        qnT_sb = per.tile([128, EC, NQC], BF16)         # normalized Q, transposed
        aoT_sb = per.tile([128, EC, NQC], BF16)         # attn out, transposed
        qp_sb = per.tile([128, NT, E], F32)             # Qp residual (natural)
        rk_pp = per.tile([128, KC], F32)                # 0.125/||k|| per key
        ones128 = per.tile([128, 1], BF16)
        onesrow = per.tile([1, 128], BF16)
        ident = per.tile([128, 128], BF16)
        eps24 = per.tile([128, 1], F32)
        epsln = per.tile([128, 1], F32)
        bk_sb = per.tile([128, EC], F32)
        if not ln_trivial:
            gam_bc = per.tile([128, E], F32)
            bet_bc = per.tile([128, E], F32)

        nc.vector.memset(ones128, 1.0)
        nc.vector.memset(onesrow, 1.0)
        make_identity(nc, ident)
        nc.vector.memset(eps24, 1e-24)
        nc.vector.memset(epsln, LN_EPS)
        # ones column (col 64) in every head's V weights -> rowsum in PV
        nc.vector.memset(v_sb[:, :, :, D:D + 1], 1.0)
        nc.sync.dma_start(out=bk_sb, in_=bk_pp)
        if not ln_trivial:
            nc.gpsimd.dma_start(out=gam_bc, in_=bcast_rows(gam, 128, E))
            nc.gpsimd.dma_start(out=bet_bc, in_=bcast_rows(bet, 128, E))

        rk_dram = dram.tile([1, NK], F32)

        # ---- load pools (V opened before K for LIFO; DMAs emitted below) -
        pv = ExitStack()
        pvp = pv.enter_context(tc.tile_pool(name="pv", bufs=1))
        vT_sb = pvp.tile([128, IC_K, NK], BF16)
        wv_sb = pvp.tile([128, IC_K, E], BF16)
        bv_sb = pvp.tile([1, E], BF16)

        # ---- phase K: kpT = (key @ Wk + bk)^T, rk = 0.125/||k|| ---------
        pk = ExitStack()
        pkp = pk.enter_context(tc.tile_pool(name="pk", bufs=1))
        pks = pk.enter_context(tc.tile_pool(name="pks", bufs=3))
        psk = pk.enter_context(tc.tile_pool(name="psk", bufs=3, space="PSUM"))
        pss = pk.enter_context(tc.tile_pool(name="pss", bufs=2, space="PSUM"))
        kT_sb = pkp.tile([128, IC_K, NK], BF16)
        wk_sb = pkp.tile([128, IC_K, E], BF16)
        ks_sb = pkp.tile([1, NK], F32)
        rk_row = pkp.tile([1, NK], F32)
        # K loads first, striped across the two HWDGE queues (sync + scalar)
        # so the K projection can start ASAP; V loads queue up behind them.
        kT_r = kT.rearrange("(c p) n -> p c n", p=128)
        wk_r = wk.rearrange("(c p) e -> p c e", p=128)
        vT_r = vT.rearrange("(c p) n -> p c n", p=128)
        wv_r = wv.rearrange("(c p) e -> p c e", p=128)
        for ic in range(IC_K):
            eng = nc.sync if ic % 2 == 0 else nc.scalar
            eng.dma_start(out=kT_sb[:, ic, :], in_=kT_r[:, ic, :])
            eng2 = nc.scalar if ic % 2 == 0 else nc.sync
            eng2.dma_start(out=wk_sb[:, ic, :], in_=wk_r[:, ic, :])
        for ic in range(IC_K):
            eng = nc.sync if ic % 2 == 0 else nc.scalar
            eng.dma_start(out=vT_sb[:, ic, :], in_=vT_r[:, ic, :])
            eng2 = nc.scalar if ic % 2 == 0 else nc.sync
            eng2.dma_start(out=wv_sb[:, ic, :], in_=wv_r[:, ic, :])
        nc.sync.dma_start(out=bv_sb, in_=bv_r)

        for j in range(4):
            ps_ss = pss.tile([1, 512], F32, tag="ps_ss")
            for ec in range(EC):
                ps_k = psk.tile([128, 512], F32, tag="ps_k")
                for ic in range(IC_K):
                    nc.tensor.matmul(ps_k,
                                     wk_sb[:, ic, ec * 128:(ec + 1) * 128],
                                     kT_sb[:, ic, j * 512:(j + 1) * 512],
                                     start=(ic == 0), stop=(ic == IC_K - 1))
                kslice = kpT_sb[:, ec, j * 512:(j + 1) * 512]
                if biases_zero:
                    nc.scalar.copy(out=kslice, in_=ps_k)
                else:
                    nc.scalar.activation(out=kslice, in_=ps_k, func=AF.Identity,
                                         bias=bk_sb[:, ec:ec + 1], scale=1.0)
                sq = pks.tile([128, 512], BF16, tag="sq")
                nc.vector.tensor_mul(out=sq, in0=kslice, in1=kslice)
                nc.tensor.matmul(ps_ss, ones128, sq,
                                 start=(ec == 0), stop=(ec == EC - 1))
            nc.vector.tensor_copy(out=ks_sb[:, j * 512:(j + 1) * 512],
                                  in_=ps_ss)
        # 8*||k|| = sqrt(64*ssq);  rk = 1/(8*||k||) = 0.125/||k||
        nc.scalar.activation(out=ks_sb, in_=ks_sb, func=AF.Sqrt,
                             bias=eps24[0:1, :], scale=64.0)
        nc.vector.reciprocal_approx_fast(out=rk_row, in_=ks_sb)
        nc.sync.dma_start(out=rk_dram, in_=rk_row)
        nc.sync.dma_start(out=rk_pp,
                          in_=rk_dram.rearrange("one (a b) -> b (one a)", b=128))

        pk.close()

        # ---- V projection chains: PSUM pool + emitter -------------------
        # g0 chains ride the Q phase's idle PE; g1 chains ride the
        # exp-bound attention phase.
        psv_ctx = ExitStack()
        psv = psv_ctx.enter_context(tc.tile_pool(name="psv", bufs=2, space="PSUM"))
        vchains = [(kc, 0) for kc in range(KC)] + [(kc, 1) for kc in range(KC)]
        vidx = [0]

        def emit_vchain():
            kc, g = vchains[vidx[0]]
            vidx[0] += 1
            ps_v = psv.tile([128, 512], F32, tag="ps_v", name=f"psv{kc}_{g}")
            for ic in range(IC_K):
                nc.tensor.matmul(ps_v,
                                 vT_sb[:, ic, kc * 128:(kc + 1) * 128],
                                 wv_sb[:, ic, g * 512:(g + 1) * 512],
                                 start=(ic == 0),
                                 stop=(biases_zero and ic == IC_K - 1))
            if not biases_zero:
                nc.tensor.matmul(ps_v, onesrow,
                                 bv_sb[:, g * 512:(g + 1) * 512],
                                 start=False, stop=True)
            nc.vector.tensor_copy(
                out=v_sb[:, kc, g * 8:(g + 1) * 8, 0:D],
                in_=ps_v.rearrange("p (h d) -> p h d", d=D))

        # ---- phase Q: loads + Qp natural (+residual) + QnT via PE -------
        # (V projection is deferred into the attention phase, where the PE
        #  has idle capacity under the exp-bound ScalarE stream.)
        pq = ExitStack()
        pqp = pq.enter_context(tc.tile_pool(name="pq", bufs=1))
        qT_sb = pqp.tile([128, IC_Q, NQC], BF16)
        wq_sb = pqp.tile([128, IC_Q, E], BF16)
        bq_sb = pqp.tile([1, E], BF16)
        qT_r = qT.rearrange("(c p) n -> p c n", p=128)
        wq_r = wq.rearrange("(c p) e -> p c e", p=128)
        for ic in range(IC_Q):
            eng = nc.sync if ic % 2 == 0 else nc.scalar
            eng.dma_start(out=qT_sb[:, ic, :], in_=qT_r[:, ic, :])
            eng2 = nc.scalar if ic % 2 == 0 else nc.sync
            eng2.dma_start(out=wq_sb[:, ic, :], in_=wq_r[:, ic, :])
        nc.sync.dma_start(out=bq_sb, in_=bq_r)

        qsc = pq.enter_context(tc.tile_pool(name="qsc", bufs=2))
        psq = pq.enter_context(tc.tile_pool(name="psq", bufs=2, space="PSUM"))
        pst = pq.enter_context(tc.tile_pool(name="pst", bufs=2, space="PSUM"))

        for nt in range(NT):
            for _ in range(4):
                emit_vchain()
            ps_q = psq.tile([128, E], F32, tag="ps_q")
            for half in range(2):
                for ic in range(IC_Q):
                    nc.tensor.matmul(ps_q[:, half * 512:(half + 1) * 512],
                                     qT_sb[:, ic, nt * 128:(nt + 1) * 128],
                                     wq_sb[:, ic, half * 512:(half + 1) * 512],
                                     start=(ic == 0),
                                     stop=(biases_zero and ic == IC_Q - 1))
                if not biases_zero:
                    nc.tensor.matmul(ps_q[:, half * 512:(half + 1) * 512],
                                     onesrow, bq_sb[:, half * 512:(half + 1) * 512],
                                     start=False, stop=True)
            qp_nt = qp_sb[:, nt, :]
            nc.scalar.copy(out=qp_nt, in_=ps_q)
            sq_q = qsc.tile([128, E], F32, tag="sqq")
            nc.vector.tensor_mul(out=sq_q, in0=qp_nt, in1=qp_nt)
            ssq = qsc.tile([128, 1], F32, tag="ssq")
            nc.vector.reduce_sum(out=ssq, in_=sq_q, axis=mybir.AxisListType.X)
            nc.scalar.activation(out=ssq, in_=ssq, func=AF.Sqrt,
                                 bias=eps24, scale=1.0)
            rq_t = qsc.tile([128, 1], F32, tag="rqt")
            nc.vector.reciprocal(out=rq_t, in_=ssq)
            qn_st = qsc.tile([128, E], BF16, tag="qnst")
            nc.scalar.mul(out=qn_st, in_=ps_q, mul=rq_t)
            for ec in range(EC):
                tp = pst.tile([128, 128], BF16, tag="tp")
                nc.tensor.transpose(tp, qn_st[:, ec * 128:(ec + 1) * 128], ident)
                nc.vector.tensor_copy(
                    out=qnT_sb[:, ec, nt * 128:(nt + 1) * 128], in_=tp)

        pq.close()

        # ---- tail input loads (overlap attention) -----------------------
        pe = ExitStack()
        pep = pe.enter_context(tc.tile_pool(name="pe", bufs=1))
        wo_sb = pep.tile([128, EC, E], BF16)
        bo_sb = pep.tile([1, E], BF16)
        nc.sync.dma_start(out=wo_sb, in_=wo.rearrange("(c p) e -> p c e", p=128))
        nc.sync.dma_start(out=bo_sb, in_=bo_r)

        # ---- attention: heads outer, key chunks inner, PSUM accumulate --
        # V projection chains are interleaved into the PE stream here: the
        # phase is ScalarE(exp)-bound, so the V matmuls ride in PE idle time.
        pa = ExitStack()
        pss_a = pa.enter_context(tc.tile_pool(name="pssa", bufs=2, space="PSUM"))
        pop = pa.enter_context(tc.tile_pool(name="pop", bufs=1, space="PSUM"))
        esp = pa.enter_context(tc.tile_pool(name="esp", bufs=3))
        rep = pa.enter_context(tc.tile_pool(name="rep", bufs=2))
        rbp = pa.enter_context(tc.tile_pool(name="rbp", bufs=2))

        def emit_qk(hp, kc):
            ps_s = pss_a.tile([128, 2 * NQC], F32, tag="ps_s")
            for i in range(2):
                nc.tensor.matmul(
                    ps_s[:, i * NQC:(i + 1) * NQC],
                    kpT_sb[i * D:(i + 1) * D, hp, kc * 128:(kc + 1) * 128],
                    qnT_sb[i * D:(i + 1) * D, hp, :],
                    start=True, stop=True)
            return ps_s

        for hp in range(HP):
            po = pop.tile([128, 2 * NQC], F32, tag="po", name=f"po{hp}")
            if hp == 4:
                while vidx[0] < 2 * KC:
                    emit_vchain()
            # QK runs two chunks ahead of exp so PE stalls (po reuse, V
            # chains) never starve the ScalarE exp stream.
            ps_list = {0: emit_qk(hp, 0), 1: emit_qk(hp, 1)}
            for kc in range(KC):
                # remaining g1 chains: one every third slot
                if hp <= 3 and (hp * KC + kc) % 3 == 0 and vidx[0] < 2 * KC:
                    emit_vchain()
                es = esp.tile([128, 2 * NQC], BF16, tag="es")
                nc.scalar.activation(out=es, in_=ps_list.pop(kc), func=AF.Exp,
                                     scale=rk_pp[:, kc:kc + 1], bias=0.0)
                for i in range(2):
                    nc.tensor.matmul(po[0:D + 1, i * NQC:(i + 1) * NQC],
                                     v_sb[:, kc, 2 * hp + i, :],
                                     es[:, i * NQC:(i + 1) * NQC],
                                     start=(kc == 0), stop=(kc == KC - 1))
                if kc + 2 < KC:
                    ps_list[kc + 2] = emit_qk(hp, kc + 2)
            # evacuate po fast (single DVE copy) so its PSUM bank frees for
            # the next head pair; normalize from the SBUF copy.
            acc_t = rep.tile([128, 2 * NQC], F32, tag="acc")
            nc.vector.tensor_copy(out=acc_t[0:D + 1, :], in_=po[0:D + 1, :])
            re_t = rep.tile([1, 2 * NQC], F32, tag="re")
            nc.vector.tensor_copy(out=re_t, in_=acc_t[D:D + 1, :])
            nc.vector.reciprocal_approx_fast(out=re_t, in_=re_t)
            rb_t = rbp.tile([D, 2 * NQC], F32, tag="rb")
            nc.gpsimd.partition_broadcast(rb_t, re_t, channels=D)
            nc.vector.tensor_mul(out=aoT_sb[0:D, hp, :],
                                 in0=acc_t[0:D, 0:NQC], in1=rb_t[:, 0:NQC])
            a1 = rep.tile([D, NQC], BF16, tag="a1")
            nc.vector.tensor_mul(out=a1, in0=acc_t[0:D, NQC:2 * NQC],
                                 in1=rb_t[:, NQC:2 * NQC])
            nc.sync.dma_start(out=aoT_sb[D:128, hp, :], in_=a1)

        # ---- phase E: out proj + residual + layernorm -------------------
        # ps_f reuses the attention score PSUM pool (those banks free as soon
        # as the last exp reads them, before the final normalize) so the
        # O-proj starts without a PSUM WAR stall and the PE stays warm.
        with tc.tile_pool(name="lnp", bufs=2) as lnp:
            for nt in range(NT):
                ps_f = pss_a.tile([128, 2 * NQC], F32, tag="ps_s")
                for half in range(2):
                    for fc in range(EC):
                        nc.tensor.matmul(ps_f[:, half * 512:(half + 1) * 512],
                                         aoT_sb[:, fc, nt * 128:(nt + 1) * 128],
                                         wo_sb[:, fc, half * 512:(half + 1) * 512],
                                         start=(fc == 0),
                                         stop=(biases_zero and fc == EC - 1))
                    if not biases_zero:
                        nc.tensor.matmul(ps_f[:, half * 512:(half + 1) * 512],
                                         onesrow,
                                         bo_sb[:, half * 512:(half + 1) * 512],
                                         start=False, stop=True)
                xs = lnp.tile([128, E], F32, tag="xs")
                nc.vector.tensor_add(out=xs, in0=ps_f[:, 0:E], in1=qp_sb[:, nt, :])
                stats = lnp.tile([128, 2, 6], F32, tag="st")
                xs3 = xs.rearrange("p (a b) -> p a b", b=512)
                for sg in range(2):
                    nc.vector.bn_stats(out=stats[:, sg, :], in_=xs3[:, sg, :])
                mv = lnp.tile([128, 2], F32, tag="mv")
                nc.vector.bn_aggr(out=mv, in_=stats)
                rstd = lnp.tile([128, 1], F32, tag="rstd")
                nc.scalar.activation(out=rstd, in_=mv[:, 1:2], func=AF.Sqrt,
                                     bias=epsln, scale=1.0)
                nc.vector.reciprocal(out=rstd, in_=rstd)
                nmr = lnp.tile([128, 1], F32, tag="nmr")
                nc.vector.tensor_mul(out=nmr, in0=mv[:, 0:1], in1=rstd)
                nc.scalar.mul(out=nmr, in_=nmr, mul=-1.0)
                ot = lnp.tile([128, E], F32, tag="ot")
                if ln_trivial:
                    nc.scalar.activation(out=ot, in_=xs, func=AF.Identity,
                                         scale=rstd, bias=nmr)
                else:
                    xn = lnp.tile([128, E], F32, tag="xn")
                    nc.scalar.activation(out=xn, in_=xs, func=AF.Identity,
                                         scale=rstd, bias=nmr)
                    nc.vector.tensor_mul(out=xn, in0=xn, in1=gam_bc)
                    nc.vector.tensor_add(out=ot, in0=xn, in1=bet_bc)
                nc.sync.dma_start(out=out[nt * 128:(nt + 1) * 128, :], in_=ot)

        pa.close()
        pe.close()
        psv_ctx.close()
        pv.close()

    nc.compile()
    return nc


_NC_CACHE = {}
_last_in_maps = None
_last_flags = (True, True)


def _get_nc(flags=None):
    if flags is None:
        flags = _last_flags
    if flags not in _NC_CACHE:
        _NC_CACHE[flags] = build(*flags)
    return _NC_CACHE[flags]


def kernel(**inputs):
    q = np.asarray(inputs["query"], np.float32)
    k = np.asarray(inputs["key"], np.float32)
    v = np.asarray(inputs["value"], np.float32)
    Wq = np.asarray(inputs["Wq"], np.float32).astype(ml_dtypes.bfloat16)
    Wk = np.asarray(inputs["Wk"], np.float32).astype(ml_dtypes.bfloat16)
    Wv = np.asarray(inputs["Wv"], np.float32).astype(ml_dtypes.bfloat16)
    Wo = np.asarray(inputs["Wo"], np.float32).astype(ml_dtypes.bfloat16)
    bq = np.asarray(inputs["bq"], np.float32)
    bk = np.asarray(inputs["bk"], np.float32)
    bv = np.asarray(inputs["bv"], np.float32)
    bo = np.asarray(inputs["bo"], np.float32)
    gam = np.asarray(inputs["ln_gamma"], np.float32)
    bet = np.asarray(inputs["ln_beta"], np.float32)

    bk_pp = np.ascontiguousarray(bk.reshape(EC, 128).T)
    bq_r = bq.reshape(1, E).astype(ml_dtypes.bfloat16)
    bv_r = bv.reshape(1, E).astype(ml_dtypes.bfloat16)
    bo_r = bo.reshape(1, E).astype(ml_dtypes.bfloat16)
    kTs = [np.ascontiguousarray(k[b].T.astype(ml_dtypes.bfloat16)) for b in range(B)]
    vTs = [np.ascontiguousarray(v[b].T.astype(ml_dtypes.bfloat16)) for b in range(B)]

    in_maps = []
    for c in range(NC):
        b, r0 = c // 4, (c % 4) * NQC
        qTa = np.ascontiguousarray(q[b, r0:r0 + NQC, :].T.astype(ml_dtypes.bfloat16))
        in_maps.append({
            "qT": qTa, "kT": kTs[b], "vT": vTs[b],
            "wq": Wq, "wk": Wk, "wv": Wv, "wo": Wo,
            "bq_r": bq_r, "bk_pp": bk_pp, "bv_r": bv_r, "bo_r": bo_r,
            "gam": gam, "bet": bet,
        })

    biases_zero = not (bq.any() or bk.any() or bv.any() or bo.any())
    ln_trivial = bool(np.all(gam == 1.0) and not bet.any())
    global _last_in_maps, _last_flags
    _last_in_maps = in_maps
    _last_flags = (biases_zero, ln_trivial)
    nc = _get_nc(_last_flags)
    res = bass_utils.run_bass_kernel_spmd(nc, in_maps, core_ids=list(range(NC)))

    out = np.empty((B, NQ, E), np.float32)
    for c in range(NC):
        b, r0 = c // 4, (c % 4) * NQC
        out[b, r0:r0 + NQC, :] = res.results[c]["out"]
    return out


# revision 26
# speedup vs baseline: 1.8876x; 1.0164x over previous
"""CrossAttention (cosine-normalized QK) Trainium2 Bass kernel, 8-core SPMD.

Sharding: batch (2) x query-row blocks (4) -> 8 cores. Each core computes the
full K/V projection for its batch (replicated within a batch group) and a
512-row slice of queries; output rows are disjoint, so the gather is a pure
concatenation (no collectives).

v4 (~320us, vs ~600us v2 baseline): engine-overlap structure.
 - Phases: K proj -> Q proj (+16 V-proj chains riding Q's idle PE) ->
   attention (+16 more V chains riding the exp-bound PE) -> O proj + LN.
 - Attention heads-outer / key-chunks-inner: PV accumulates all 16 key
   chunks in PSUM; QK emitted two chunks ahead of exp so PE stalls never
   starve ScalarE; exp stream runs ScalarE at ~100% (147us floor).
 - po is single-buffered; it is evacuated to SBUF with one DVE copy so the
   bank frees fast; softmax denominators via reciprocal_approx_fast (DVE)
   + gpsimd partition_broadcast (no DRAM roundtrip).
 - Biases folded into matmuls (ones-row trick) or ACT bias operand; PSUM->
   SBUF moves on whichever of ScalarE/DVE is idle in that phase; qnT via
   PE transpose; Qp residual kept in SBUF; input loads striped across both
   HWDGE queues with K's inputs first; O proj PSUM reuses the score pool
   (avoids a WAR stall + HAM cold-clock penalty at the tail).
"""

import numpy as np
import ml_dtypes
from contextlib import ExitStack

import concourse.bacc as bacc
import concourse.bass as bass
import concourse.mybir as mybir
import concourse.tile as tile
from concourse import bass_utils
from concourse.masks import make_identity

F32 = mybir.dt.float32
BF16 = mybir.dt.bfloat16
AF = mybir.ActivationFunctionType

B, NQ, NK = 2, 2048, 2048
QD, KD, E, H = 1024, 768, 1024, 16
D = E // H          # 64
NC = 8              # cores
NQC = NQ * B // NC  # 512 query rows per core
SCALE = D ** -0.5   # 0.125
LN_EPS = 1e-5

IC_Q = QD // 128    # 8  contraction chunks for Q proj
IC_K = KD // 128    # 6  contraction chunks for K/V proj
EC = E // 128       # 8  embed chunks
KC = NK // 128      # 16 key chunks
NT = NQC // 128     # 4  query-row tiles
HP = H // 2         # 8  head pairs


def build(biases_zero=False, ln_trivial=False):
    nc = bacc.Bacc("TRN2", target_bir_lowering=False, debug=False,
                   enable_asserts=False, num_devices=1)

    qT = nc.dram_tensor("qT", [QD, NQC], BF16, kind="ExternalInput").ap()
    kT = nc.dram_tensor("kT", [KD, NK], BF16, kind="ExternalInput").ap()
    vT = nc.dram_tensor("vT", [KD, NK], BF16, kind="ExternalInput").ap()
    wq = nc.dram_tensor("wq", [QD, E], BF16, kind="ExternalInput").ap()
    wk = nc.dram_tensor("wk", [KD, E], BF16, kind="ExternalInput").ap()
    wv = nc.dram_tensor("wv", [KD, E], BF16, kind="ExternalInput").ap()
    wo = nc.dram_tensor("wo", [E, E], BF16, kind="ExternalInput").ap()
    bq_r = nc.dram_tensor("bq_r", [1, E], BF16, kind="ExternalInput").ap()
    bv_r = nc.dram_tensor("bv_r", [1, E], BF16, kind="ExternalInput").ap()
    bo_r = nc.dram_tensor("bo_r", [1, E], BF16, kind="ExternalInput").ap()
    bk_pp = nc.dram_tensor("bk_pp", [128, EC], F32, kind="ExternalInput").ap()
    gam = nc.dram_tensor("gam", [E], F32, kind="ExternalInput").ap()
    bet = nc.dram_tensor("bet", [E], F32, kind="ExternalInput").ap()
    out = nc.dram_tensor("out", [NQC, E], F32, kind="ExternalOutput").ap()

    def bcast_rows(src_ap, parts, n):
        return bass.AP(tensor=src_ap.tensor, offset=src_ap.offset,
                       ap=[[0, parts], [1, n]])

    with tile.TileContext(nc) as tc, ExitStack() as ctx:
        # ---- persistent tiles -------------------------------------------
        per = ctx.enter_context(tc.tile_pool(name="per", bufs=1))
        dram = ctx.enter_context(tc.tile_pool(name="dram", bufs=1, space="DRAM"))

        kpT_sb = per.tile([128, EC, NK], BF16)          # K proj, transposed
        v_sb = per.tile([128, KC, H, D + 1], BF16)      # V + ones col per head
        qnT_sb = per.tile([128, EC, NQC], BF16)         # normalized Q, transposed
        aoT_sb = per.tile([128, EC, NQC], BF16)         # attn out, transposed
        qp_sb = per.tile([128, NT, E], F32)             # Qp residual (natural)
        rk_pp = per.tile([128, KC], F32)                # 0.125/||k|| per key
        ones128 = per.tile([128, 1], BF16)
        onesrow = per.tile([1, 128], BF16)
        ident = per.tile([128, 128], BF16)
        eps24 = per.tile([128, 1], F32)
        epsln = per.tile([128, 1], F32)
        bk_sb = per.tile([128, EC], F32)
        if not ln_trivial:
            gam_bc = per.tile([128, E], F32)
            bet_bc = per.tile([128, E], F32)

        nc.vector.memset(ones128, 1.0)
        nc.vector.memset(onesrow, 1.0)
        make_identity(nc, ident)
        nc.vector.memset(eps24, 1e-24)
        nc.vector.memset(epsln, LN_EPS)
        # ones column (col 64) in every head's V weights -> rowsum in PV
        nc.vector.memset(v_sb[:, :, :, D:D + 1], 1.0)
        nc.sync.dma_start(out=bk_sb, in_=bk_pp)
        if not ln_trivial:
            nc.gpsimd.dma_start(out=gam_bc, in_=bcast_rows(gam, 128, E))
            nc.gpsimd.dma_start(out=bet_bc, in_=bcast_rows(bet, 128, E))

        rk_dram = dram.tile([1, NK], F32)

        # ---- load pools (V opened before K for LIFO; DMAs emitted below) -
        pv = ExitStack()
        pvp = pv.enter_context(tc.tile_pool(name="pv", bufs=1))
        vT_sb = pvp.tile([128, IC_K, NK], BF16)
        wv_sb = pvp.tile([128, IC_K, E], BF16)
        bv_sb = pvp.tile([1, E], BF16)

        # ---- phase K: kpT = (key @ Wk + bk)^T, rk = 0.125/||k|| ---------
        pk = ExitStack()
        pkp = pk.enter_context(tc.tile_pool(name="pk", bufs=1))
        pks = pk.enter_context(tc.tile_pool(name="pks", bufs=3))
        psk = pk.enter_context(tc.tile_pool(name="psk", bufs=3, space="PSUM"))
        pss = pk.enter_context(tc.tile_pool(name="pss", bufs=2, space="PSUM"))
        kT_sb = pkp.tile([128, IC_K, NK], BF16)
        wk_sb = pkp.tile([128, IC_K, E], BF16)
        ks_sb = pkp.tile([1, NK], F32)
        rk_row = pkp.tile([1, NK], F32)
        # K loads first, striped across the two HWDGE queues (sync + scalar)
        # so the K projection can start ASAP; V loads queue up behind them.
        kT_r = kT.rearrange("(c p) n -> p c n", p=128)
        wk_r = wk.rearrange("(c p) e -> p c e", p=128)
        vT_r = vT.rearrange("(c p) n -> p c n", p=128)
        wv_r = wv.rearrange("(c p) e -> p c e", p=128)
        for ic in range(IC_K):
            eng = nc.sync if ic % 2 == 0 else nc.scalar
            eng.dma_start(out=kT_sb[:, ic, :], in_=kT_r[:, ic, :])
            eng2 = nc.scalar if ic % 2 == 0 else nc.sync
            eng2.dma_start(out=wk_sb[:, ic, :], in_=wk_r[:, ic, :])
        for ic in range(IC_K):
            eng = nc.sync if ic % 2 == 0 else nc.scalar
            eng.dma_start(out=vT_sb[:, ic, :], in_=vT_r[:, ic, :])
            eng2 = nc.scalar if ic % 2 == 0 else nc.sync
            eng2.dma_start(out=wv_sb[:, ic, :], in_=wv_r[:, ic, :])
        nc.sync.dma_start(out=bv_sb, in_=bv_r)

        for j in range(4):
            ps_ss = pss.tile([1, 512], F32, tag="ps_ss")
            for ec in range(EC):
                ps_k = psk.tile([128, 512], F32, tag="ps_k")
                for ic in range(IC_K):
                    nc.tensor.matmul(ps_k,
                                     wk_sb[:, ic, ec * 128:(ec + 1) * 128],
                                     kT_sb[:, ic, j * 512:(j + 1) * 512],
                                     start=(ic == 0), stop=(ic == IC_K - 1))
                kslice = kpT_sb[:, ec, j * 512:(j + 1) * 512]
                if biases_zero:
                    nc.scalar.copy(out=kslice, in_=ps_k)
                else:
                    nc.scalar.activation(out=kslice, in_=ps_k, func=AF.Identity,
                                         bias=bk_sb[:, ec:ec + 1], scale=1.0)
                sq = pks.tile([128, 512], BF16, tag="sq")
                nc.vector.tensor_mul(out=sq, in0=kslice, in1=kslice)
                nc.tensor.matmul(ps_ss, ones128, sq,
                                 start=(ec == 0), stop=(ec == EC - 1))
            nc.vector.tensor_copy(out=ks_sb[:, j * 512:(j + 1) * 512],
                                  in_=ps_ss)
        # 8*||k|| = sqrt(64*ssq);  rk = 1/(8*||k||) = 0.125/||k||
        nc.scalar.activation(out=ks_sb, in_=ks_sb, func=AF.Sqrt,
                             bias=eps24[0:1, :], scale=64.0)
        nc.vector.reciprocal_approx_fast(out=rk_row, in_=ks_sb)
        nc.sync.dma_start(out=rk_dram, in_=rk_row)
        nc.sync.dma_start(out=rk_pp,
                          in_=rk_dram.rearrange("one (a b) -> b (one a)", b=128))

        pk.close()

        # ---- V projection chains: PSUM pool + emitter -------------------
        # g0 chains ride the Q phase's idle PE; g1 chains ride the
        # exp-bound attention phase.
        psv_ctx = ExitStack()
        psv = psv_ctx.enter_context(tc.tile_pool(name="psv", bufs=2, space="PSUM"))
        vchains = [(kc, 0) for kc in range(KC)] + [(kc, 1) for kc in range(KC)]
        vidx = [0]

        def emit_vchain():
            kc, g = vchains[vidx[0]]
            vidx[0] += 1
            ps_v = psv.tile([128, 512], F32, tag="ps_v", name=f"psv{kc}_{g}")
            for ic in range(IC_K):
                nc.tensor.matmul(ps_v,
                                 vT_sb[:, ic, kc * 128:(kc + 1) * 128],
                                 wv_sb[:, ic, g * 512:(g + 1) * 512],
                                 start=(ic == 0),
                                 stop=(biases_zero and ic == IC_K - 1))
            if not biases_zero:
                nc.tensor.matmul(ps_v, onesrow,
                                 bv_sb[:, g * 512:(g + 1) * 512],
                                 start=False, stop=True)
            nc.vector.tensor_copy(
                out=v_sb[:, kc, g * 8:(g + 1) * 8, 0:D],
                in_=ps_v.rearrange("p (h d) -> p h d", d=D))

        # ---- phase Q: loads + Qp natural (+residual) + QnT via PE -------
        # (V projection is deferred into the attention phase, where the PE
        #  has idle capacity under the exp-bound ScalarE stream.)
        pq = ExitStack()
        pqp = pq.enter_context(tc.tile_pool(name="pq", bufs=1))
        qT_sb = pqp.tile([128, IC_Q, NQC], BF16)
        wq_sb = pqp.tile([128, IC_Q, E], BF16)
        bq_sb = pqp.tile([1, E], BF16)
        qT_r = qT.rearrange("(c p) n -> p c n", p=128)
        wq_r = wq.rearrange("(c p) e -> p c e", p=128)
        for ic in range(IC_Q):
            eng = nc.sync if ic % 2 == 0 else nc.scalar
            eng.dma_start(out=qT_sb[:, ic, :], in_=qT_r[:, ic, :])
            eng2 = nc.scalar if ic % 2 == 0 else nc.sync
            eng2.dma_start(out=wq_sb[:, ic, :], in_=wq_r[:, ic, :])
        nc.sync.dma_start(out=bq_sb, in_=bq_r)

        qsc = pq.enter_context(tc.tile_pool(name="qsc", bufs=2))
        psq = pq.enter_context(tc.tile_pool(name="psq", bufs=2, space="PSUM"))
        pst = pq.enter_context(tc.tile_pool(name="pst", bufs=2, space="PSUM"))

        for nt in range(NT):
            for _ in range(4):
                emit_vchain()
            ps_q = psq.tile([128, E], F32, tag="ps_q")
            for half in range(2):
                for ic in range(IC_Q):
                    nc.tensor.matmul(ps_q[:, half * 512:(half + 1) * 512],
                                     qT_sb[:, ic, nt * 128:(nt + 1) * 128],
                                     wq_sb[:, ic, half * 512:(half + 1) * 512],
                                     start=(ic == 0),
                                     stop=(biases_zero and ic == IC_Q - 1))
                if not biases_zero:
                    nc.tensor.matmul(ps_q[:, half * 512:(half + 1) * 512],
                                     onesrow, bq_sb[:, half * 512:(half + 1) * 512],
                                     start=False, stop=True)
            qp_nt = qp_sb[:, nt, :]
            nc.scalar.copy(out=qp_nt, in_=ps_q)
            sq_q = qsc.tile([128, E], F32, tag="sqq")
            ssq = qsc.tile([128, 1], F32, tag="ssq")
            nc.scalar.activation(out=sq_q, in_=ps_q, func=AF.Square,
                                 accum_out=ssq)
            nc.scalar.activation(out=ssq, in_=ssq, func=AF.Sqrt,
                                 bias=eps24, scale=1.0)
            rq_t = qsc.tile([128, 1], F32, tag="rqt")
            nc.vector.reciprocal(out=rq_t, in_=ssq)
            qn_st = qsc.tile([128, E], BF16, tag="qnst")
            nc.scalar.mul(out=qn_st, in_=ps_q, mul=rq_t)
            for ec in range(EC):
                tp = pst.tile([128, 128], BF16, tag="tp")
                nc.tensor.transpose(tp, qn_st[:, ec * 128:(ec + 1) * 128], ident)
                nc.vector.tensor_copy(
                    out=qnT_sb[:, ec, nt * 128:(nt + 1) * 128], in_=tp)

        pq.close()

        # ---- tail input loads (overlap attention) -----------------------
        pe = ExitStack()
        pep = pe.enter_context(tc.tile_pool(name="pe", bufs=1))
        wo_sb = pep.tile([128, EC, E], BF16)
        bo_sb = pep.tile([1, E], BF16)
        nc.sync.dma_start(out=wo_sb, in_=wo.rearrange("(c p) e -> p c e", p=128))
        nc.sync.dma_start(out=bo_sb, in_=bo_r)

        # ---- attention: heads outer, key chunks inner, PSUM accumulate --
        # V projection chains are interleaved into the PE stream here: the
        # phase is ScalarE(exp)-bound, so the V matmuls ride in PE idle time.
        pa = ExitStack()
        pss_a = pa.enter_context(tc.tile_pool(name="pssa", bufs=2, space="PSUM"))
        pop = pa.enter_context(tc.tile_pool(name="pop", bufs=1, space="PSUM"))
        esp = pa.enter_context(tc.tile_pool(name="esp", bufs=3))
        rep = pa.enter_context(tc.tile_pool(name="rep", bufs=2))
        rbp = pa.enter_context(tc.tile_pool(name="rbp", bufs=2))

        def emit_qk(hp, kc):
            ps_s = pss_a.tile([128, 2 * NQC], F32, tag="ps_s")
            for i in range(2):
                nc.tensor.matmul(
                    ps_s[:, i * NQC:(i + 1) * NQC],
                    kpT_sb[i * D:(i + 1) * D, hp, kc * 128:(kc + 1) * 128],
                    qnT_sb[i * D:(i + 1) * D, hp, :],
                    start=True, stop=True)
            return ps_s

        for hp in range(HP):
            po = pop.tile([128, 2 * NQC], F32, tag="po", name=f"po{hp}")
            if hp == 4:
                while vidx[0] < 2 * KC:
                    emit_vchain()
            # QK runs two chunks ahead of exp so PE stalls (po reuse, V
            # chains) never starve the ScalarE exp stream.
            ps_list = {0: emit_qk(hp, 0), 1: emit_qk(hp, 1)}
            for kc in range(KC):
                # remaining g1 chains: one every third slot
                if hp <= 3 and (hp * KC + kc) % 3 == 0 and vidx[0] < 2 * KC:
                    emit_vchain()
                es = esp.tile([128, 2 * NQC], BF16, tag="es")
                nc.scalar.activation(out=es, in_=ps_list.pop(kc), func=AF.Exp,
                                     scale=rk_pp[:, kc:kc + 1], bias=0.0)
                for i in range(2):
                    nc.tensor.matmul(po[0:D + 1, i * NQC:(i + 1) * NQC],
                                     v_sb[:, kc, 2 * hp + i, :],
                                     es[:, i * NQC:(i + 1) * NQC],
                                     start=(kc == 0), stop=(kc == KC - 1))
                if kc + 2 < KC:
                    ps_list[kc + 2] = emit_qk(hp, kc + 2)
            # evacuate po fast (single DVE copy) so its PSUM bank frees for
            # the next head pair; normalize from the SBUF copy.
            acc_t = rep.tile([128, 2 * NQC], F32, tag="acc")
            nc.vector.tensor_copy(out=acc_t[0:D + 1, :], in_=po[0:D + 1, :])
            re_t = rep.tile([1, 2 * NQC], F32, tag="re")
            nc.vector.tensor_copy(out=re_t, in_=acc_t[D:D + 1, :])
            nc.vector.reciprocal_approx_fast(out=re_t, in_=re_t)
            rb_t = rbp.tile([D, 2 * NQC], F32, tag="rb")
            nc.gpsimd.partition_broadcast(rb_t, re_t, channels=D)
            nc.vector.tensor_mul(out=aoT_sb[0:D, hp, :],
                                 in0=acc_t[0:D, 0:NQC], in1=rb_t[:, 0:NQC])
            a1 = rep.tile([D, NQC], BF16, tag="a1")
            nc.vector.tensor_mul(out=a1, in0=acc_t[0:D, NQC:2 * NQC],
                                 in1=rb_t[:, NQC:2 * NQC])
            nc.sync.dma_start(out=aoT_sb[D:128, hp, :], in_=a1)

        # ---- phase E: out proj + residual + layernorm -------------------
        # ps_f reuses the attention score PSUM pool (those banks free as soon
        # as the last exp reads them, before the final normalize) so the
        # O-proj starts without a PSUM WAR stall and the PE stays warm.
        with tc.tile_pool(name="lnp", bufs=2) as lnp:
            for nt in range(NT):
                ps_f = pss_a.tile([128, 2 * NQC], F32, tag="ps_s")
                for half in range(2):
                    for fc in range(EC):
                        nc.tensor.matmul(ps_f[:, half * 512:(half + 1) * 512],
                                         aoT_sb[:, fc, nt * 128:(nt + 1) * 128],
                                         wo_sb[:, fc, half * 512:(half + 1) * 512],
                                         start=(fc == 0),
                                         stop=(biases_zero and fc == EC - 1))
                    if not biases_zero:
                        nc.tensor.matmul(ps_f[:, half * 512:(half + 1) * 512],
                                         onesrow,
                                         bo_sb[:, half * 512:(half + 1) * 512],
                                         start=False, stop=True)
                xs = lnp.tile([128, E], F32, tag="xs")
                nc.vector.tensor_add(out=xs, in0=ps_f[:, 0:E], in1=qp_sb[:, nt, :])
                stats = lnp.tile([128, 2, 6], F32, tag="st")
                xs3 = xs.rearrange("p (a b) -> p a b", b=512)
                for sg in range(2):
                    nc.vector.bn_stats(out=stats[:, sg, :], in_=xs3[:, sg, :])
                mv = lnp.tile([128, 2], F32, tag="mv")
                nc.vector.bn_aggr(out=mv, in_=stats)
                rstd = lnp.tile([128, 1], F32, tag="rstd")
                nc.scalar.activation(out=rstd, in_=mv[:, 1:2], func=AF.Sqrt,
                                     bias=epsln, scale=1.0)
                nc.vector.reciprocal(out=rstd, in_=rstd)
                nmr = lnp.tile([128, 1], F32, tag="nmr")
                nc.vector.scalar_tensor_tensor(
                    out=nmr, in0=mv[:, 0:1], scalar=-1.0, in1=rstd,
                    op0=mybir.AluOpType.mult, op1=mybir.AluOpType.mult)
                ot = lnp.tile([128, E], F32, tag="ot")
                if ln_trivial:
                    nc.scalar.activation(out=ot, in_=xs, func=AF.Identity,
                                         scale=rstd, bias=nmr)
                else:
                    xn = lnp.tile([128, E], F32, tag="xn")
                    nc.scalar.activation(out=xn, in_=xs, func=AF.Identity,
                                         scale=rstd, bias=nmr)
                    nc.vector.tensor_mul(out=xn, in0=xn, in1=gam_bc)
                    nc.vector.tensor_add(out=ot, in0=xn, in1=bet_bc)
                nc.sync.dma_start(out=out[nt * 128:(nt + 1) * 128, :], in_=ot)

        pa.close()
        pe.close()
        psv_ctx.close()
        pv.close()

    nc.compile()
    return nc


_NC_CACHE = {}
_last_in_maps = None
_last_flags = (True, True)


def _get_nc(flags=None):
    if flags is None:
        flags = _last_flags
    if flags not in _NC_CACHE:
        _NC_CACHE[flags] = build(*flags)
    return _NC_CACHE[flags]


def kernel(**inputs):
    q = np.asarray(inputs["query"], np.float32)
    k = np.asarray(inputs["key"], np.float32)
    v = np.asarray(inputs["value"], np.float32)
    Wq = np.asarray(inputs["Wq"], np.float32).astype(ml_dtypes.bfloat16)
    Wk = np.asarray(inputs["Wk"], np.float32).astype(ml_dtypes.bfloat16)
    Wv = np.asarray(inputs["Wv"], np.float32).astype(ml_dtypes.bfloat16)
    Wo = np.asarray(inputs["Wo"], np.float32).astype(ml_dtypes.bfloat16)
    bq = np.asarray(inputs["bq"], np.float32)
    bk = np.asarray(inputs["bk"], np.float32)
    bv = np.asarray(inputs["bv"], np.float32)
    bo = np.asarray(inputs["bo"], np.float32)
    gam = np.asarray(inputs["ln_gamma"], np.float32)
    bet = np.asarray(inputs["ln_beta"], np.float32)

    bk_pp = np.ascontiguousarray(bk.reshape(EC, 128).T)
    bq_r = bq.reshape(1, E).astype(ml_dtypes.bfloat16)
    bv_r = bv.reshape(1, E).astype(ml_dtypes.bfloat16)
    bo_r = bo.reshape(1, E).astype(ml_dtypes.bfloat16)
    kTs = [np.ascontiguousarray(k[b].T.astype(ml_dtypes.bfloat16)) for b in range(B)]
    vTs = [np.ascontiguousarray(v[b].T.astype(ml_dtypes.bfloat16)) for b in range(B)]

    in_maps = []
    for c in range(NC):
        b, r0 = c // 4, (c % 4) * NQC
        qTa = np.ascontiguousarray(q[b, r0:r0 + NQC, :].T.astype(ml_dtypes.bfloat16))
        in_maps.append({
            "qT": qTa, "kT": kTs[b], "vT": vTs[b],
            "wq": Wq, "wk": Wk, "wv": Wv, "wo": Wo,
            "bq_r": bq_r, "bk_pp": bk_pp, "bv_r": bv_r, "bo_r": bo_r,
            "gam": gam, "bet": bet,
        })

    biases_zero = not (bq.any() or bk.any() or bv.any() or bo.any())
    ln_trivial = bool(np.all(gam == 1.0) and not bet.any())
    global _last_in_maps, _last_flags
    _last_in_maps = in_maps
    _last_flags = (biases_zero, ln_trivial)
    nc = _get_nc(_last_flags)
    res = bass_utils.run_bass_kernel_spmd(nc, in_maps, core_ids=list(range(NC)))

    out = np.empty((B, NQ, E), np.float32)
    for c in range(NC):
        b, r0 = c // 4, (c % 4) * NQC
        out[b, r0:r0 + NQC, :] = res.results[c]["out"]
    return out


# revision 28
# speedup vs baseline: 1.9096x; 1.0116x over previous
"""CrossAttention (cosine-normalized QK) Trainium2 Bass kernel, 8-core SPMD.

Sharding: batch (2) x query-row blocks (4) -> 8 cores. Each core computes the
full K/V projection for its batch (replicated within a batch group) and a
512-row slice of queries; output rows are disjoint, so the gather is a pure
concatenation (no collectives).

v4 (~320us, vs ~600us v2 baseline): engine-overlap structure.
 - Phases: K proj -> Q proj (+16 V-proj chains riding Q's idle PE) ->
   attention (+16 more V chains riding the exp-bound PE) -> O proj + LN.
 - Attention heads-outer / key-chunks-inner: PV accumulates all 16 key
   chunks in PSUM; QK emitted two chunks ahead of exp so PE stalls never
   starve ScalarE; exp stream runs ScalarE at ~100% (147us floor).
 - po is single-buffered; it is evacuated to SBUF with one DVE copy so the
   bank frees fast; softmax denominators via reciprocal_approx_fast (DVE)
   + gpsimd partition_broadcast (no DRAM roundtrip).
 - Biases folded into matmuls (ones-row trick) or ACT bias operand; PSUM->
   SBUF moves on whichever of ScalarE/DVE is idle in that phase; qnT via
   PE transpose; Qp residual kept in SBUF; input loads striped across both
   HWDGE queues with K's inputs first; O proj PSUM reuses the score pool
   (avoids a WAR stall + HAM cold-clock penalty at the tail).
"""

import numpy as np
import ml_dtypes
from contextlib import ExitStack

import concourse.bacc as bacc
import concourse.bass as bass
import concourse.mybir as mybir
import concourse.tile as tile
from concourse import bass_utils
from concourse.masks import make_identity

F32 = mybir.dt.float32
BF16 = mybir.dt.bfloat16
AF = mybir.ActivationFunctionType

B, NQ, NK = 2, 2048, 2048
QD, KD, E, H = 1024, 768, 1024, 16
D = E // H          # 64
NC = 8              # cores
NQC = NQ * B // NC  # 512 query rows per core
SCALE = D ** -0.5   # 0.125
LN_EPS = 1e-5

IC_Q = QD // 128    # 8  contraction chunks for Q proj
IC_K = KD // 128    # 6  contraction chunks for K/V proj
EC = E // 128       # 8  embed chunks
KC = NK // 128      # 16 key chunks
NT = NQC // 128     # 4  query-row tiles
HP = H // 2         # 8  head pairs


def build(biases_zero=False, ln_trivial=False):
    nc = bacc.Bacc("TRN2", target_bir_lowering=False, debug=False,
                   enable_asserts=False, num_devices=1)

    qT = nc.dram_tensor("qT", [QD, NQC], BF16, kind="ExternalInput").ap()
    kT = nc.dram_tensor("kT", [KD, NK], BF16, kind="ExternalInput").ap()
    vT = nc.dram_tensor("vT", [KD, NK], BF16, kind="ExternalInput").ap()
    wq = nc.dram_tensor("wq", [QD, E], BF16, kind="ExternalInput").ap()
    wk = nc.dram_tensor("wk", [KD, E], BF16, kind="ExternalInput").ap()
    wv = nc.dram_tensor("wv", [KD, E], BF16, kind="ExternalInput").ap()
    wo = nc.dram_tensor("wo", [E, E], BF16, kind="ExternalInput").ap()
    bq_r = nc.dram_tensor("bq_r", [1, E], BF16, kind="ExternalInput").ap()
    bv_r = nc.dram_tensor("bv_r", [1, E], BF16, kind="ExternalInput").ap()
    bo_r = nc.dram_tensor("bo_r", [1, E], BF16, kind="ExternalInput").ap()
    bk_pp = nc.dram_tensor("bk_pp", [128, EC], F32, kind="ExternalInput").ap()
    gam = nc.dram_tensor("gam", [E], F32, kind="ExternalInput").ap()
    bet = nc.dram_tensor("bet", [E], F32, kind="ExternalInput").ap()
    out = nc.dram_tensor("out", [NQC, E], F32, kind="ExternalOutput").ap()

    def bcast_rows(src_ap, parts, n):
        return bass.AP(tensor=src_ap.tensor, offset=src_ap.offset,
                       ap=[[0, parts], [1, n]])

    with tile.TileContext(nc) as tc, ExitStack() as ctx:
        # ---- persistent tiles -------------------------------------------
        per = ctx.enter_context(tc.tile_pool(name="per", bufs=1))
        dram = ctx.enter_context(tc.tile_pool(name="dram", bufs=1, space="DRAM"))

        kpT_sb = per.tile([128, EC, NK], BF16)          # K proj, transposed
        v_sb = per.tile([128, KC, H, D + 1], BF16)      # V + ones col per head
        qnT_sb = per.tile([128, EC, NQC], BF16)         # normalized Q, transposed
        aoT_sb = per.tile([128, EC, NQC], BF16)         # attn out, transposed
        qp_sb = per.tile([128, NT, E], F32)             # Qp residual (natural)
        rk_pp = per.tile([128, KC], F32)                # 0.125/||k|| per key
        ones128 = per.tile([128, 1], BF16)
        onesrow = per.tile([1, 128], BF16)
        ident = per.tile([128, 128], BF16)
        eps24 = per.tile([128, 1], F32)
        epsln = per.tile([128, 1], F32)
        bk_sb = per.tile([128, EC], F32)
        if not ln_trivial:
            gam_bc = per.tile([128, E], F32)
            bet_bc = per.tile([128, E], F32)

        nc.vector.memset(ones128, 1.0)
        nc.vector.memset(onesrow, 1.0)
        make_identity(nc, ident)
        nc.vector.memset(eps24, 1e-24)
        nc.vector.memset(epsln, LN_EPS)
        # ones column (col 64) in every head's V weights -> rowsum in PV
        nc.vector.memset(v_sb[:, :, :, D:D + 1], 1.0)
        nc.sync.dma_start(out=bk_sb, in_=bk_pp)
        if not ln_trivial:
            nc.gpsimd.dma_start(out=gam_bc, in_=bcast_rows(gam, 128, E))
            nc.gpsimd.dma_start(out=bet_bc, in_=bcast_rows(bet, 128, E))

        rk_dram = dram.tile([1, NK], F32)

        # ---- load pools (V opened before K for LIFO; DMAs emitted below) -
        pv = ExitStack()
        pvp = pv.enter_context(tc.tile_pool(name="pv", bufs=1))
        vT_sb = pvp.tile([128, IC_K, NK], BF16)
        wv_sb = pvp.tile([128, IC_K, E], BF16)
        bv_sb = pvp.tile([1, E], BF16)

        # ---- phase K: kpT = (key @ Wk + bk)^T, rk = 0.125/||k|| ---------
        pk = ExitStack()
        pkp = pk.enter_context(tc.tile_pool(name="pk", bufs=1))
        pks = pk.enter_context(tc.tile_pool(name="pks", bufs=3))
        psk = pk.enter_context(tc.tile_pool(name="psk", bufs=3, space="PSUM"))
        pss = pk.enter_context(tc.tile_pool(name="pss", bufs=2, space="PSUM"))
        kT_sb = pkp.tile([128, IC_K, NK], BF16)
        wk_sb = pkp.tile([128, IC_K, E], BF16)
        ks_sb = pkp.tile([1, NK], F32)
        rk_row = pkp.tile([1, NK], F32)
        # K loads first, striped across the two HWDGE queues (sync + scalar)
        # so the K projection can start ASAP; V loads queue up behind them.
        kT_r = kT.rearrange("(c p) n -> p c n", p=128)
        wk_r = wk.rearrange("(c p) e -> p c e", p=128)
        vT_r = vT.rearrange("(c p) n -> p c n", p=128)
        wv_r = wv.rearrange("(c p) e -> p c e", p=128)
        for ic in range(IC_K):
            eng = nc.sync if ic % 2 == 0 else nc.scalar
            eng.dma_start(out=kT_sb[:, ic, :], in_=kT_r[:, ic, :])
            eng2 = nc.scalar if ic % 2 == 0 else nc.sync
            eng2.dma_start(out=wk_sb[:, ic, :], in_=wk_r[:, ic, :])
        for ic in range(IC_K):
            eng = nc.sync if ic % 2 == 0 else nc.scalar
            eng.dma_start(out=vT_sb[:, ic, :], in_=vT_r[:, ic, :])
            eng2 = nc.scalar if ic % 2 == 0 else nc.sync
            eng2.dma_start(out=wv_sb[:, ic, :], in_=wv_r[:, ic, :])
        nc.sync.dma_start(out=bv_sb, in_=bv_r)

        for j in range(4):
            ps_ss = pss.tile([1, 512], F32, tag="ps_ss")
            for ec in range(EC):
                ps_k = psk.tile([128, 512], F32, tag="ps_k")
                for ic in range(IC_K):
                    nc.tensor.matmul(ps_k,
                                     wk_sb[:, ic, ec * 128:(ec + 1) * 128],
                                     kT_sb[:, ic, j * 512:(j + 1) * 512],
                                     start=(ic == 0), stop=(ic == IC_K - 1))
                kslice = kpT_sb[:, ec, j * 512:(j + 1) * 512]
                if biases_zero:
                    nc.scalar.copy(out=kslice, in_=ps_k)
                else:
                    nc.scalar.activation(out=kslice, in_=ps_k, func=AF.Identity,
                                         bias=bk_sb[:, ec:ec + 1], scale=1.0)
                sq = pks.tile([128, 512], BF16, tag="sq")
                nc.vector.tensor_mul(out=sq, in0=kslice, in1=kslice)
                nc.tensor.matmul(ps_ss, ones128, sq,
                                 start=(ec == 0), stop=(ec == EC - 1))
            nc.vector.tensor_copy(out=ks_sb[:, j * 512:(j + 1) * 512],
                                  in_=ps_ss)
        # 8*||k|| = sqrt(64*ssq);  rk = 1/(8*||k||) = 0.125/||k||
        nc.scalar.activation(out=ks_sb, in_=ks_sb, func=AF.Sqrt,
                             bias=eps24[0:1, :], scale=64.0)
        nc.vector.reciprocal_approx_fast(out=rk_row, in_=ks_sb)
        nc.sync.dma_start(out=rk_dram, in_=rk_row)
        nc.sync.dma_start(out=rk_pp,
                          in_=rk_dram.rearrange("one (a b) -> b (one a)", b=128))

        pk.close()

        # ---- V projection chains: PSUM pool + emitter -------------------
        # g0 chains ride the Q phase's idle PE; g1 chains ride the
        # exp-bound attention phase.
        psv_ctx = ExitStack()
        psv = psv_ctx.enter_context(tc.tile_pool(name="psv", bufs=2, space="PSUM"))
        vchains = [(kc, 0) for kc in range(KC)] + [(kc, 1) for kc in range(KC)]
        vidx = [0]

        def emit_vchain():
            kc, g = vchains[vidx[0]]
            vidx[0] += 1
            ps_v = psv.tile([128, 512], F32, tag="ps_v", name=f"psv{kc}_{g}")
            for ic in range(IC_K):
                nc.tensor.matmul(ps_v,
                                 vT_sb[:, ic, kc * 128:(kc + 1) * 128],
                                 wv_sb[:, ic, g * 512:(g + 1) * 512],
                                 start=(ic == 0),
                                 stop=(biases_zero and ic == IC_K - 1))
            if not biases_zero:
                nc.tensor.matmul(ps_v, onesrow,
                                 bv_sb[:, g * 512:(g + 1) * 512],
                                 start=False, stop=True)
            nc.vector.tensor_copy(
                out=v_sb[:, kc, g * 8:(g + 1) * 8, 0:D],
                in_=ps_v.rearrange("p (h d) -> p h d", d=D))

        # ---- phase Q: loads + Qp natural (+residual) + QnT via PE -------
        # (V projection is deferred into the attention phase, where the PE
        #  has idle capacity under the exp-bound ScalarE stream.)
        pq = ExitStack()
        pqp = pq.enter_context(tc.tile_pool(name="pq", bufs=1))
        qT_sb = pqp.tile([128, IC_Q, NQC], BF16)
        wq_sb = pqp.tile([128, IC_Q, E], BF16)
        bq_sb = pqp.tile([1, E], BF16)
        qT_r = qT.rearrange("(c p) n -> p c n", p=128)
        wq_r = wq.rearrange("(c p) e -> p c e", p=128)
        for ic in range(IC_Q):
            eng = nc.sync if ic % 2 == 0 else nc.scalar
            eng.dma_start(out=qT_sb[:, ic, :], in_=qT_r[:, ic, :])
            eng2 = nc.scalar if ic % 2 == 0 else nc.sync
            eng2.dma_start(out=wq_sb[:, ic, :], in_=wq_r[:, ic, :])
        nc.sync.dma_start(out=bq_sb, in_=bq_r)

        qsc = pq.enter_context(tc.tile_pool(name="qsc", bufs=2))
        psq = pq.enter_context(tc.tile_pool(name="psq", bufs=2, space="PSUM"))
        pst = pq.enter_context(tc.tile_pool(name="pst", bufs=2, space="PSUM"))

        for nt in range(NT):
            for _ in range(4):
                emit_vchain()
            ps_q = psq.tile([128, E], F32, tag="ps_q")
            for half in range(2):
                for ic in range(IC_Q):
                    nc.tensor.matmul(ps_q[:, half * 512:(half + 1) * 512],
                                     qT_sb[:, ic, nt * 128:(nt + 1) * 128],
                                     wq_sb[:, ic, half * 512:(half + 1) * 512],
                                     start=(ic == 0),
                                     stop=(biases_zero and ic == IC_Q - 1))
                if not biases_zero:
                    nc.tensor.matmul(ps_q[:, half * 512:(half + 1) * 512],
                                     onesrow, bq_sb[:, half * 512:(half + 1) * 512],
                                     start=False, stop=True)
            qp_nt = qp_sb[:, nt, :]
            nc.scalar.copy(out=qp_nt, in_=ps_q)
            sq_q = qsc.tile([128, E], F32, tag="sqq")
            ssq = qsc.tile([128, 1], F32, tag="ssq")
            nc.scalar.activation(out=sq_q, in_=ps_q, func=AF.Square,
                                 accum_out=ssq)
            nc.scalar.activation(out=ssq, in_=ssq, func=AF.Sqrt,
                                 bias=eps24, scale=1.0)
            rq_t = qsc.tile([128, 1], F32, tag="rqt")
            nc.vector.reciprocal(out=rq_t, in_=ssq)
            qn_st = qsc.tile([128, E], BF16, tag="qnst")
            nc.scalar.mul(out=qn_st, in_=ps_q, mul=rq_t)
            for ec in range(EC):
                tp = pst.tile([128, 128], BF16, tag="tp")
                nc.tensor.transpose(tp, qn_st[:, ec * 128:(ec + 1) * 128], ident)
                nc.vector.tensor_copy(
                    out=qnT_sb[:, ec, nt * 128:(nt + 1) * 128], in_=tp)

        pq.close()

        # ---- tail input loads (overlap attention) -----------------------
        pe = ExitStack()
        pep = pe.enter_context(tc.tile_pool(name="pe", bufs=1))
        wo_sb = pep.tile([128, EC, E], BF16)
        bo_sb = pep.tile([1, E], BF16)
        nc.sync.dma_start(out=wo_sb, in_=wo.rearrange("(c p) e -> p c e", p=128))
        nc.sync.dma_start(out=bo_sb, in_=bo_r)

        # ---- attention: heads outer, key chunks inner, PSUM accumulate --
        # V projection chains are interleaved into the PE stream here: the
        # phase is ScalarE(exp)-bound, so the V matmuls ride in PE idle time.
        pa = ExitStack()
        pss_a = pa.enter_context(tc.tile_pool(name="pssa", bufs=2, space="PSUM"))
        pop = pa.enter_context(tc.tile_pool(name="pop", bufs=1, space="PSUM"))
        esp = pa.enter_context(tc.tile_pool(name="esp", bufs=3))
        rep = pa.enter_context(tc.tile_pool(name="rep", bufs=2))
        rbp = pa.enter_context(tc.tile_pool(name="rbp", bufs=2))

        def emit_qk(hp, kc):
            ps_s = pss_a.tile([128, 2 * NQC], F32, tag="ps_s")
            for i in range(2):
                nc.tensor.matmul(
                    ps_s[:, i * NQC:(i + 1) * NQC],
                    kpT_sb[i * D:(i + 1) * D, hp, kc * 128:(kc + 1) * 128],
                    qnT_sb[i * D:(i + 1) * D, hp, :],
                    start=True, stop=True)
            return ps_s

        for hp in range(HP):
            po = pop.tile([128, 2 * NQC], F32, tag="po", name=f"po{hp}")
            if hp == 4:
                while vidx[0] < 2 * KC:
                    emit_vchain()
            # QK runs two chunks ahead of exp so PE stalls (po reuse, V
            # chains) never starve the ScalarE exp stream.
            ps_list = {0: emit_qk(hp, 0), 1: emit_qk(hp, 1)}
            for kc in range(KC):
                # remaining g1 chains: one every third slot
                if hp <= 3 and (hp * KC + kc) % 3 == 0 and vidx[0] < 2 * KC:
                    emit_vchain()
                es = esp.tile([128, 2 * NQC], BF16, tag="es")
                nc.scalar.activation(out=es, in_=ps_list.pop(kc), func=AF.Exp,
                                     scale=rk_pp[:, kc:kc + 1], bias=0.0)
                for i in range(2):
                    nc.tensor.matmul(po[0:D + 1, i * NQC:(i + 1) * NQC],
                                     v_sb[:, kc, 2 * hp + i, :],
                                     es[:, i * NQC:(i + 1) * NQC],
                                     start=(kc == 0), stop=(kc == KC - 1))
                if kc + 2 < KC:
                    ps_list[kc + 2] = emit_qk(hp, kc + 2)
            # evacuate po fast (single DVE copy) so its PSUM bank frees for
            # the next head pair; normalize from the SBUF copy.
            acc_t = rep.tile([128, 2 * NQC], F32, tag="acc")
            nc.vector.tensor_copy(out=acc_t[0:D + 1, :], in_=po[0:D + 1, :])
            re_t = rep.tile([1, 2 * NQC], F32, tag="re")
            nc.vector.tensor_copy(out=re_t, in_=acc_t[D:D + 1, :])
            nc.vector.reciprocal_approx_fast(out=re_t, in_=re_t)
            rb_t = rbp.tile([D, 2 * NQC], F32, tag="rb")
            nc.gpsimd.partition_broadcast(rb_t, re_t, channels=D)
            nc.vector.tensor_mul(out=aoT_sb[0:D, hp, :],
                                 in0=acc_t[0:D, 0:NQC], in1=rb_t[:, 0:NQC])
            a1 = rep.tile([D, NQC], BF16, tag="a1")
            nc.vector.tensor_mul(out=a1, in0=acc_t[0:D, NQC:2 * NQC],
                                 in1=rb_t[:, NQC:2 * NQC])
            nc.sync.dma_start(out=aoT_sb[D:128, hp, :], in_=a1)

        # ---- phase E: out proj + residual + layernorm -------------------
        # ps_f reuses the attention score PSUM pool (those banks free as soon
        # as the last exp reads them, before the final normalize) so the
        # O-proj starts without a PSUM WAR stall and the PE stays warm.
        with tc.tile_pool(name="lnp", bufs=2) as lnp:
            for nt in range(NT):
                ps_f = pss_a.tile([128, 2 * NQC], F32, tag="ps_s")
                for half in range(2):
                    for fc in range(EC):
                        nc.tensor.matmul(ps_f[:, half * 512:(half + 1) * 512],
                                         aoT_sb[:, fc, nt * 128:(nt + 1) * 128],
                                         wo_sb[:, fc, half * 512:(half + 1) * 512],
                                         start=(fc == 0),
                                         stop=(biases_zero and fc == EC - 1))
                    if not biases_zero:
                        nc.tensor.matmul(ps_f[:, half * 512:(half + 1) * 512],
                                         onesrow,
                                         bo_sb[:, half * 512:(half + 1) * 512],
                                         start=False, stop=True)
                xs = lnp.tile([128, E], F32, tag="xs")
                nc.vector.tensor_add(out=xs, in0=ps_f[:, 0:E], in1=qp_sb[:, nt, :])
                stats = lnp.tile([128, 2, 6], F32, tag="st")
                xs3 = xs.rearrange("p (a b) -> p a b", b=512)
                for sg in range(2):
                    nc.vector.bn_stats(out=stats[:, sg, :], in_=xs3[:, sg, :])
                mv = lnp.tile([128, 2], F32, tag="mv")
                nc.vector.bn_aggr(out=mv, in_=stats)
                rstd = lnp.tile([128, 1], F32, tag="rstd")
                nc.scalar.activation(out=rstd, in_=mv[:, 1:2], func=AF.Sqrt,
                                     bias=epsln, scale=1.0)
                nc.vector.reciprocal(out=rstd, in_=rstd)
                nmr = lnp.tile([128, 1], F32, tag="nmr")
                nc.vector.scalar_tensor_tensor(
                    out=nmr, in0=mv[:, 0:1], scalar=-1.0, in1=rstd,
                    op0=mybir.AluOpType.mult, op1=mybir.AluOpType.mult)
                ot = lnp.tile([128, E], F32, tag="ot")
                if ln_trivial:
                    nc.scalar.activation(out=ot, in_=xs, func=AF.Identity,
                                         scale=rstd, bias=nmr)
                else:
                    xn = lnp.tile([128, E], F32, tag="xn")
                    nc.scalar.activation(out=xn, in_=xs, func=AF.Identity,
                                         scale=rstd, bias=nmr)
                    nc.vector.tensor_mul(out=xn, in0=xn, in1=gam_bc)
                    nc.vector.tensor_add(out=ot, in0=xn, in1=bet_bc)
                nc.sync.dma_start(out=out[nt * 128:(nt + 1) * 128, :], in_=ot)

        pa.close()
        pe.close()
        psv_ctx.close()
        pv.close()

    nc.compile()
    return nc


_NC_CACHE = {}
_last_in_maps = None
_last_flags = (True, True)


def _get_nc(flags=None):
    if flags is None:
        flags = _last_flags
    if flags not in _NC_CACHE:
        _NC_CACHE[flags] = build(*flags)
    return _NC_CACHE[flags]


def kernel(**inputs):
    q = np.asarray(inputs["query"], np.float32)
    k = np.asarray(inputs["key"], np.float32)
    v = np.asarray(inputs["value"], np.float32)
    Wq = np.asarray(inputs["Wq"], np.float32).astype(ml_dtypes.bfloat16)
    Wk = np.asarray(inputs["Wk"], np.float32).astype(ml_dtypes.bfloat16)
    Wv = np.asarray(inputs["Wv"], np.float32).astype(ml_dtypes.bfloat16)
    Wo = np.asarray(inputs["Wo"], np.float32).astype(ml_dtypes.bfloat16)
    bq = np.asarray(inputs["bq"], np.float32)
    bk = np.asarray(inputs["bk"], np.float32)
    bv = np.asarray(inputs["bv"], np.float32)
    bo = np.asarray(inputs["bo"], np.float32)
    gam = np.asarray(inputs["ln_gamma"], np.float32)
    bet = np.asarray(inputs["ln_beta"], np.float32)

    bk_pp = np.ascontiguousarray(bk.reshape(EC, 128).T)
    bq_r = bq.reshape(1, E).astype(ml_dtypes.bfloat16)
    bv_r = bv.reshape(1, E).astype(ml_dtypes.bfloat16)
    bo_r = bo.reshape(1, E).astype(ml_dtypes.bfloat16)
    kTs = [np.ascontiguousarray(k[b].T.astype(ml_dtypes.bfloat16)) for b in range(B)]
    vTs = [np.ascontiguousarray(v[b].T.astype(ml_dtypes.bfloat16)) for b in range(B)]

    in_maps = []
    for c in range(NC):
        b, r0 = c // 4, (c % 4) * NQC
        qTa = np.ascontiguousarray(q[b, r0:r0 + NQC, :].T.astype(ml_dtypes.bfloat16))
        in_maps.append({
            "qT": qTa, "kT": kTs[b], "vT": vTs[b],
            "wq": Wq, "wk": Wk, "wv": Wv, "wo": Wo,
            "bq_r": bq_r, "bk_pp": bk_pp, "bv_r": bv_r, "bo_r": bo_r,
            "gam": gam, "bet": bet,
        })

    biases_zero = not (bq.any() or bk.any() or bv.any() or bo.any())
    ln_trivial = bool(np.all(gam == 1.0) and not bet.any())
    global _last_in_maps, _last_flags
    _last_in_maps = in_maps
    _last_flags = (biases_zero, ln_trivial)
    nc = _get_nc(_last_flags)
    res = bass_utils.run_bass_kernel_spmd(nc, in_maps, core_ids=list(range(NC)))

    out = np.empty((B, NQ, E), np.float32)
    for c in range(NC):
        b, r0 = c // 4, (c % 4) * NQC
        out[b, r0:r0 + NQC, :] = res.results[c]["out"]
    return out


# revision 29
# speedup vs baseline: 1.9212x; 1.0061x over previous
"""CrossAttention (cosine-normalized QK) Trainium2 Bass kernel, 8-core SPMD.

Sharding: batch (2) x query-row blocks (4) -> 8 cores. Each core computes the
full K/V projection for its batch (replicated within a batch group) and a
512-row slice of queries; output rows are disjoint, so the gather is a pure
concatenation (no collectives).

v4 (~320us, vs ~600us v2 baseline): engine-overlap structure.
 - Phases: K proj -> Q proj (+16 V-proj chains riding Q's idle PE) ->
   attention (+16 more V chains riding the exp-bound PE) -> O proj + LN.
 - Attention heads-outer / key-chunks-inner: PV accumulates all 16 key
   chunks in PSUM; QK emitted two chunks ahead of exp so PE stalls never
   starve ScalarE; exp stream runs ScalarE at ~100% (147us floor).
 - po is single-buffered; it is evacuated to SBUF with one DVE copy so the
   bank frees fast; softmax denominators via reciprocal_approx_fast (DVE)
   + gpsimd partition_broadcast (no DRAM roundtrip).
 - Biases folded into matmuls (ones-row trick) or ACT bias operand; PSUM->
   SBUF moves on whichever of ScalarE/DVE is idle in that phase; qnT via
   PE transpose; Qp residual kept in SBUF; input loads striped across both
   HWDGE queues with K's inputs first; O proj PSUM reuses the score pool
   (avoids a WAR stall + HAM cold-clock penalty at the tail).
"""

import numpy as np
import ml_dtypes
from contextlib import ExitStack

import concourse.bacc as bacc
import concourse.bass as bass
import concourse.mybir as mybir
import concourse.tile as tile
from concourse import bass_utils
from concourse.masks import make_identity

F32 = mybir.dt.float32
BF16 = mybir.dt.bfloat16
AF = mybir.ActivationFunctionType

B, NQ, NK = 2, 2048, 2048
QD, KD, E, H = 1024, 768, 1024, 16
D = E // H          # 64
NC = 8              # cores
NQC = NQ * B // NC  # 512 query rows per core
SCALE = D ** -0.5   # 0.125
LN_EPS = 1e-5

IC_Q = QD // 128    # 8  contraction chunks for Q proj
IC_K = KD // 128    # 6  contraction chunks for K/V proj
EC = E // 128       # 8  embed chunks
KC = NK // 128      # 16 key chunks
NT = NQC // 128     # 4  query-row tiles
HP = H // 2         # 8  head pairs


def build(biases_zero=False, ln_trivial=False):
    nc = bacc.Bacc("TRN2", target_bir_lowering=False, debug=False,
                   enable_asserts=False, num_devices=1)

    qT = nc.dram_tensor("qT", [QD, NQC], BF16, kind="ExternalInput").ap()
    kT = nc.dram_tensor("kT", [KD, NK], BF16, kind="ExternalInput").ap()
    vT = nc.dram_tensor("vT", [KD, NK], BF16, kind="ExternalInput").ap()
    wq = nc.dram_tensor("wq", [QD, E], BF16, kind="ExternalInput").ap()
    wk = nc.dram_tensor("wk", [KD, E], BF16, kind="ExternalInput").ap()
    wv = nc.dram_tensor("wv", [KD, E], BF16, kind="ExternalInput").ap()
    wo = nc.dram_tensor("wo", [E, E], BF16, kind="ExternalInput").ap()
    bq_r = nc.dram_tensor("bq_r", [1, E], BF16, kind="ExternalInput").ap()
    bv_r = nc.dram_tensor("bv_r", [1, E], BF16, kind="ExternalInput").ap()
    bo_r = nc.dram_tensor("bo_r", [1, E], BF16, kind="ExternalInput").ap()
    bk_pp = nc.dram_tensor("bk_pp", [128, EC], F32, kind="ExternalInput").ap()
    gam = nc.dram_tensor("gam", [E], F32, kind="ExternalInput").ap()
    bet = nc.dram_tensor("bet", [E], F32, kind="ExternalInput").ap()
    out = nc.dram_tensor("out", [NQC, E], F32, kind="ExternalOutput").ap()

    def bcast_rows(src_ap, parts, n):
        return bass.AP(tensor=src_ap.tensor, offset=src_ap.offset,
                       ap=[[0, parts], [1, n]])

    with tile.TileContext(nc) as tc, ExitStack() as ctx:
        # ---- persistent tiles -------------------------------------------
        per = ctx.enter_context(tc.tile_pool(name="per", bufs=1))
        dram = ctx.enter_context(tc.tile_pool(name="dram", bufs=1, space="DRAM"))

        kpT_sb = per.tile([128, EC, NK], BF16)          # K proj, transposed
        v_sb = per.tile([128, KC, H, D + 1], BF16)      # V + ones col per head
        qnT_sb = per.tile([128, EC, NQC], BF16)         # normalized Q, transposed
        aoT_sb = per.tile([128, EC, NQC], BF16)         # attn out, transposed
        qp_sb = per.tile([128, NT, E], F32)             # Qp residual (natural)
        rk_pp = per.tile([128, KC], F32)                # 0.125/||k|| per key
        ones128 = per.tile([128, 1], BF16)
        onesrow = per.tile([1, 128], BF16)
        ident = per.tile([128, 128], BF16)
        eps24 = per.tile([128, 1], F32)
        epsln = per.tile([128, 1], F32)
        bk_sb = per.tile([128, EC], F32)
        if not ln_trivial:
            gam_bc = per.tile([128, E], F32)
            bet_bc = per.tile([128, E], F32)

        nc.vector.memset(ones128, 1.0)
        nc.vector.memset(onesrow, 1.0)
        make_identity(nc, ident)
        nc.vector.memset(eps24, 1e-24)
        nc.vector.memset(epsln, LN_EPS)
        # ones column (col 64) in every head's V weights -> rowsum in PV
        nc.vector.memset(v_sb[:, :, :, D:D + 1], 1.0)
        nc.sync.dma_start(out=bk_sb, in_=bk_pp)
        if not ln_trivial:
            nc.gpsimd.dma_start(out=gam_bc, in_=bcast_rows(gam, 128, E))
            nc.gpsimd.dma_start(out=bet_bc, in_=bcast_rows(bet, 128, E))

        rk_dram = dram.tile([1, NK], F32)

        # ---- load pools (V opened before K for LIFO; DMAs emitted below) -
        pv = ExitStack()
        pvp = pv.enter_context(tc.tile_pool(name="pv", bufs=1))
        vT_sb = pvp.tile([128, IC_K, NK], BF16)
        wv_sb = pvp.tile([128, IC_K, E], BF16)
        bv_sb = pvp.tile([1, E], BF16)

        # ---- phase K: kpT = (key @ Wk + bk)^T, rk = 0.125/||k|| ---------
        pk = ExitStack()
        pkp = pk.enter_context(tc.tile_pool(name="pk", bufs=1))
        pks = pk.enter_context(tc.tile_pool(name="pks", bufs=3))
        psk = pk.enter_context(tc.tile_pool(name="psk", bufs=3, space="PSUM"))
        pss = pk.enter_context(tc.tile_pool(name="pss", bufs=2, space="PSUM"))
        kT_sb = pkp.tile([128, IC_K, NK], BF16)
        wk_sb = pkp.tile([128, IC_K, E], BF16)
        ks_sb = pkp.tile([1, NK], F32)
        rk_row = pkp.tile([1, NK], F32)
        # K loads first, striped across the two HWDGE queues (sync + scalar)
        # so the K projection can start ASAP; V loads queue up behind them.
        kT_r = kT.rearrange("(c p) n -> p c n", p=128)
        wk_r = wk.rearrange("(c p) e -> p c e", p=128)
        vT_r = vT.rearrange("(c p) n -> p c n", p=128)
        wv_r = wv.rearrange("(c p) e -> p c e", p=128)
        for ic in range(IC_K):
            eng = nc.sync if ic % 2 == 0 else nc.scalar
            eng.dma_start(out=kT_sb[:, ic, :], in_=kT_r[:, ic, :])
            eng2 = nc.scalar if ic % 2 == 0 else nc.sync
            eng2.dma_start(out=wk_sb[:, ic, :], in_=wk_r[:, ic, :])
        for ic in range(IC_K):
            eng = nc.sync if ic % 2 == 0 else nc.scalar
            eng.dma_start(out=vT_sb[:, ic, :], in_=vT_r[:, ic, :])
            eng2 = nc.scalar if ic % 2 == 0 else nc.sync
            eng2.dma_start(out=wv_sb[:, ic, :], in_=wv_r[:, ic, :])
        nc.sync.dma_start(out=bv_sb, in_=bv_r)

        for j in range(4):
            ps_ss = pss.tile([1, 512], F32, tag="ps_ss")
            for ec in range(EC):
                ps_k = psk.tile([128, 512], F32, tag="ps_k")
                for ic in range(IC_K):
                    nc.tensor.matmul(ps_k,
                                     wk_sb[:, ic, ec * 128:(ec + 1) * 128],
                                     kT_sb[:, ic, j * 512:(j + 1) * 512],
                                     start=(ic == 0), stop=(ic == IC_K - 1))
                kslice = kpT_sb[:, ec, j * 512:(j + 1) * 512]
                if biases_zero:
                    nc.scalar.copy(out=kslice, in_=ps_k)
                else:
                    nc.scalar.activation(out=kslice, in_=ps_k, func=AF.Identity,
                                         bias=bk_sb[:, ec:ec + 1], scale=1.0)
                sq = pks.tile([128, 512], BF16, tag="sq")
                nc.vector.tensor_mul(out=sq, in0=kslice, in1=kslice)
                nc.tensor.matmul(ps_ss, ones128, sq,
                                 start=(ec == 0), stop=(ec == EC - 1))
            nc.vector.tensor_copy(out=ks_sb[:, j * 512:(j + 1) * 512],
                                  in_=ps_ss)
        # 8*||k|| = sqrt(64*ssq);  rk = 1/(8*||k||) = 0.125/||k||
        nc.scalar.activation(out=ks_sb, in_=ks_sb, func=AF.Sqrt,
                             bias=eps24[0:1, :], scale=64.0)
        nc.vector.reciprocal_approx_fast(out=rk_row, in_=ks_sb)
        nc.sync.dma_start(out=rk_dram, in_=rk_row)
        nc.sync.dma_start(out=rk_pp,
                          in_=rk_dram.rearrange("one (a b) -> b (one a)", b=128))

        pk.close()

        # ---- V projection chains: PSUM pool + emitter -------------------
        # g0 chains ride the Q phase's idle PE; g1 chains ride the
        # exp-bound attention phase.
        psv_ctx = ExitStack()
        psv = psv_ctx.enter_context(tc.tile_pool(name="psv", bufs=2, space="PSUM"))
        vchains = [(kc, 0) for kc in range(KC)] + [(kc, 1) for kc in range(KC)]
        vidx = [0]

        def emit_vchain():
            kc, g = vchains[vidx[0]]
            vidx[0] += 1
            ps_v = psv.tile([128, 512], F32, tag="ps_v", name=f"psv{kc}_{g}")
            for ic in range(IC_K):
                nc.tensor.matmul(ps_v,
                                 vT_sb[:, ic, kc * 128:(kc + 1) * 128],
                                 wv_sb[:, ic, g * 512:(g + 1) * 512],
                                 start=(ic == 0),
                                 stop=(biases_zero and ic == IC_K - 1))
            if not biases_zero:
                nc.tensor.matmul(ps_v, onesrow,
                                 bv_sb[:, g * 512:(g + 1) * 512],
                                 start=False, stop=True)
            nc.vector.tensor_copy(
                out=v_sb[:, kc, g * 8:(g + 1) * 8, 0:D],
                in_=ps_v.rearrange("p (h d) -> p h d", d=D))

        # ---- phase Q: loads + Qp natural (+residual) + QnT via PE -------
        # (V projection is deferred into the attention phase, where the PE
        #  has idle capacity under the exp-bound ScalarE stream.)
        pq = ExitStack()
        pqp = pq.enter_context(tc.tile_pool(name="pq", bufs=1))
        qT_sb = pqp.tile([128, IC_Q, NQC], BF16)
        wq_sb = pqp.tile([128, IC_Q, E], BF16)
        bq_sb = pqp.tile([1, E], BF16)
        qT_r = qT.rearrange("(c p) n -> p c n", p=128)
        wq_r = wq.rearrange("(c p) e -> p c e", p=128)
        for ic in range(IC_Q):
            eng = nc.sync if ic % 2 == 0 else nc.scalar
            eng.dma_start(out=qT_sb[:, ic, :], in_=qT_r[:, ic, :])
            eng2 = nc.scalar if ic % 2 == 0 else nc.sync
            eng2.dma_start(out=wq_sb[:, ic, :], in_=wq_r[:, ic, :])
        nc.sync.dma_start(out=bq_sb, in_=bq_r)

        qsc = pq.enter_context(tc.tile_pool(name="qsc", bufs=2))
        psq = pq.enter_context(tc.tile_pool(name="psq", bufs=2, space="PSUM"))
        pst = pq.enter_context(tc.tile_pool(name="pst", bufs=2, space="PSUM"))

        for nt in range(NT):
            ps_q = psq.tile([128, E], F32, tag="ps_q")
            for half in range(2):
                for ic in range(IC_Q):
                    nc.tensor.matmul(ps_q[:, half * 512:(half + 1) * 512],
                                     qT_sb[:, ic, nt * 128:(nt + 1) * 128],
                                     wq_sb[:, ic, half * 512:(half + 1) * 512],
                                     start=(ic == 0),
                                     stop=(biases_zero and ic == IC_Q - 1))
                if not biases_zero:
                    nc.tensor.matmul(ps_q[:, half * 512:(half + 1) * 512],
                                     onesrow, bq_sb[:, half * 512:(half + 1) * 512],
                                     start=False, stop=True)
            for _ in range(4):
                emit_vchain()
            qp_nt = qp_sb[:, nt, :]
            nc.scalar.copy(out=qp_nt, in_=ps_q)
            sq_q = qsc.tile([128, E], F32, tag="sqq")
            ssq = qsc.tile([128, 1], F32, tag="ssq")
            nc.scalar.activation(out=sq_q, in_=ps_q, func=AF.Square,
                                 accum_out=ssq)
            nc.scalar.activation(out=ssq, in_=ssq, func=AF.Sqrt,
                                 bias=eps24, scale=1.0)
            rq_t = qsc.tile([128, 1], F32, tag="rqt")
            nc.vector.reciprocal(out=rq_t, in_=ssq)
            qn_st = qsc.tile([128, E], BF16, tag="qnst")
            nc.scalar.mul(out=qn_st, in_=ps_q, mul=rq_t)
            for ec in range(EC):
                tp = pst.tile([128, 128], BF16, tag="tp")
                nc.tensor.transpose(tp, qn_st[:, ec * 128:(ec + 1) * 128], ident)
                nc.vector.tensor_copy(
                    out=qnT_sb[:, ec, nt * 128:(nt + 1) * 128], in_=tp)

        pq.close()

        # ---- tail input loads (overlap attention) -----------------------
        pe = ExitStack()
        pep = pe.enter_context(tc.tile_pool(name="pe", bufs=1))
        wo_sb = pep.tile([128, EC, E], BF16)
        bo_sb = pep.tile([1, E], BF16)
        nc.sync.dma_start(out=wo_sb, in_=wo.rearrange("(c p) e -> p c e", p=128))
        nc.sync.dma_start(out=bo_sb, in_=bo_r)

        # ---- attention: heads outer, key chunks inner, PSUM accumulate --
        # V projection chains are interleaved into the PE stream here: the
        # phase is ScalarE(exp)-bound, so the V matmuls ride in PE idle time.
        pa = ExitStack()
        pss_a = pa.enter_context(tc.tile_pool(name="pssa", bufs=2, space="PSUM"))
        pop = pa.enter_context(tc.tile_pool(name="pop", bufs=1, space="PSUM"))
        esp = pa.enter_context(tc.tile_pool(name="esp", bufs=3))
        rep = pa.enter_context(tc.tile_pool(name="rep", bufs=2))
        rbp = pa.enter_context(tc.tile_pool(name="rbp", bufs=2))

        def emit_qk(hp, kc):
            ps_s = pss_a.tile([128, 2 * NQC], F32, tag="ps_s")
            for i in range(2):
                nc.tensor.matmul(
                    ps_s[:, i * NQC:(i + 1) * NQC],
                    kpT_sb[i * D:(i + 1) * D, hp, kc * 128:(kc + 1) * 128],
                    qnT_sb[i * D:(i + 1) * D, hp, :],
                    start=True, stop=True)
            return ps_s

        for hp in range(HP):
            po = pop.tile([128, 2 * NQC], F32, tag="po", name=f"po{hp}")
            if hp == 4:
                while vidx[0] < 2 * KC:
                    emit_vchain()
            # QK runs two chunks ahead of exp so PE stalls (po reuse, V
            # chains) never starve the ScalarE exp stream.
            ps_list = {0: emit_qk(hp, 0), 1: emit_qk(hp, 1)}
            for kc in range(KC):
                es = esp.tile([128, 2 * NQC], BF16, tag="es")
                nc.scalar.activation(out=es, in_=ps_list.pop(kc), func=AF.Exp,
                                     scale=rk_pp[:, kc:kc + 1], bias=0.0)
                for i in range(2):
                    nc.tensor.matmul(po[0:D + 1, i * NQC:(i + 1) * NQC],
                                     v_sb[:, kc, 2 * hp + i, :],
                                     es[:, i * NQC:(i + 1) * NQC],
                                     start=(kc == 0), stop=(kc == KC - 1))
                if kc + 2 < KC:
                    ps_list[kc + 2] = emit_qk(hp, kc + 2)
                # g1 chains ride at the slot tail, behind the QK lookahead
                if hp <= 3 and (hp * KC + kc) % 3 == 0 and vidx[0] < 2 * KC:
                    emit_vchain()
            # evacuate po fast (single DVE copy) so its PSUM bank frees for
            # the next head pair; normalize from the SBUF copy.
            acc_t = rep.tile([128, 2 * NQC], F32, tag="acc")
            nc.vector.tensor_copy(out=acc_t[0:D + 1, :], in_=po[0:D + 1, :])
            re_t = rep.tile([1, 2 * NQC], F32, tag="re")
            nc.vector.tensor_copy(out=re_t, in_=acc_t[D:D + 1, :])
            nc.vector.reciprocal_approx_fast(out=re_t, in_=re_t)
            rb_t = rbp.tile([D, 2 * NQC], F32, tag="rb")
            nc.gpsimd.partition_broadcast(rb_t, re_t, channels=D)
            nc.vector.tensor_mul(out=aoT_sb[0:D, hp, :],
                                 in0=acc_t[0:D, 0:NQC], in1=rb_t[:, 0:NQC])
            a1 = rep.tile([D, NQC], BF16, tag="a1")
            nc.vector.tensor_mul(out=a1, in0=acc_t[0:D, NQC:2 * NQC],
                                 in1=rb_t[:, NQC:2 * NQC])
            nc.sync.dma_start(out=aoT_sb[D:128, hp, :], in_=a1)

        # ---- phase E: out proj + residual + layernorm -------------------
        # ps_f reuses the attention score PSUM pool (those banks free as soon
        # as the last exp reads them, before the final normalize) so the
        # O-proj starts without a PSUM WAR stall and the PE stays warm.
        with tc.tile_pool(name="lnp", bufs=2) as lnp:
            for nt in range(NT):
                ps_f = pss_a.tile([128, 2 * NQC], F32, tag="ps_s")
                for half in range(2):
                    for fc in range(EC):
                        nc.tensor.matmul(ps_f[:, half * 512:(half + 1) * 512],
                                         aoT_sb[:, fc, nt * 128:(nt + 1) * 128],
                                         wo_sb[:, fc, half * 512:(half + 1) * 512],
                                         start=(fc == 0),
                                         stop=(biases_zero and fc == EC - 1))
                    if not biases_zero:
                        nc.tensor.matmul(ps_f[:, half * 512:(half + 1) * 512],
                                         onesrow,
                                         bo_sb[:, half * 512:(half + 1) * 512],
                                         start=False, stop=True)
                xs = lnp.tile([128, E], F32, tag="xs")
                nc.vector.tensor_add(out=xs, in0=ps_f[:, 0:E], in1=qp_sb[:, nt, :])
                stats = lnp.tile([128, 2, 6], F32, tag="st")
                xs3 = xs.rearrange("p (a b) -> p a b", b=512)
                for sg in range(2):
                    nc.vector.bn_stats(out=stats[:, sg, :], in_=xs3[:, sg, :])
                mv = lnp.tile([128, 2], F32, tag="mv")
                nc.vector.bn_aggr(out=mv, in_=stats)
                rstd = lnp.tile([128, 1], F32, tag="rstd")
                nc.scalar.activation(out=rstd, in_=mv[:, 1:2], func=AF.Sqrt,
                                     bias=epsln, scale=1.0)
                nc.vector.reciprocal(out=rstd, in_=rstd)
                nmr = lnp.tile([128, 1], F32, tag="nmr")
                nc.vector.scalar_tensor_tensor(
                    out=nmr, in0=mv[:, 0:1], scalar=-1.0, in1=rstd,
                    op0=mybir.AluOpType.mult, op1=mybir.AluOpType.mult)
                ot = lnp.tile([128, E], F32, tag="ot")
                if ln_trivial:
                    nc.scalar.activation(out=ot, in_=xs, func=AF.Identity,
                                         scale=rstd, bias=nmr)
                else:
                    xn = lnp.tile([128, E], F32, tag="xn")
                    nc.scalar.activation(out=xn, in_=xs, func=AF.Identity,
                                         scale=rstd, bias=nmr)
                    nc.vector.tensor_mul(out=xn, in0=xn, in1=gam_bc)
                    nc.vector.tensor_add(out=ot, in0=xn, in1=bet_bc)
                nc.sync.dma_start(out=out[nt * 128:(nt + 1) * 128, :], in_=ot)

        pa.close()
        pe.close()
        psv_ctx.close()
        pv.close()

    nc.compile()
    return nc


_NC_CACHE = {}
_last_in_maps = None
_last_flags = (True, True)


def _get_nc(flags=None):
    if flags is None:
        flags = _last_flags
    if flags not in _NC_CACHE:
        _NC_CACHE[flags] = build(*flags)
    return _NC_CACHE[flags]


def kernel(**inputs):
    q = np.asarray(inputs["query"], np.float32)
    k = np.asarray(inputs["key"], np.float32)
    v = np.asarray(inputs["value"], np.float32)
    Wq = np.asarray(inputs["Wq"], np.float32).astype(ml_dtypes.bfloat16)
    Wk = np.asarray(inputs["Wk"], np.float32).astype(ml_dtypes.bfloat16)
    Wv = np.asarray(inputs["Wv"], np.float32).astype(ml_dtypes.bfloat16)
    Wo = np.asarray(inputs["Wo"], np.float32).astype(ml_dtypes.bfloat16)
    bq = np.asarray(inputs["bq"], np.float32)
    bk = np.asarray(inputs["bk"], np.float32)
    bv = np.asarray(inputs["bv"], np.float32)
    bo = np.asarray(inputs["bo"], np.float32)
    gam = np.asarray(inputs["ln_gamma"], np.float32)
    bet = np.asarray(inputs["ln_beta"], np.float32)

    bk_pp = np.ascontiguousarray(bk.reshape(EC, 128).T)
    bq_r = bq.reshape(1, E).astype(ml_dtypes.bfloat16)
    bv_r = bv.reshape(1, E).astype(ml_dtypes.bfloat16)
    bo_r = bo.reshape(1, E).astype(ml_dtypes.bfloat16)
    kTs = [np.ascontiguousarray(k[b].T.astype(ml_dtypes.bfloat16)) for b in range(B)]
    vTs = [np.ascontiguousarray(v[b].T.astype(ml_dtypes.bfloat16)) for b in range(B)]

    in_maps = []
    for c in range(NC):
        b, r0 = c // 4, (c % 4) * NQC
        qTa = np.ascontiguousarray(q[b, r0:r0 + NQC, :].T.astype(ml_dtypes.bfloat16))
        in_maps.append({
            "qT": qTa, "kT": kTs[b], "vT": vTs[b],
            "wq": Wq, "wk": Wk, "wv": Wv, "wo": Wo,
            "bq_r": bq_r, "bk_pp": bk_pp, "bv_r": bv_r, "bo_r": bo_r,
            "gam": gam, "bet": bet,
        })

    biases_zero = not (bq.any() or bk.any() or bv.any() or bo.any())
    ln_trivial = bool(np.all(gam == 1.0) and not bet.any())
    global _last_in_maps, _last_flags
    _last_in_maps = in_maps
    _last_flags = (biases_zero, ln_trivial)
    nc = _get_nc(_last_flags)
    res = bass_utils.run_bass_kernel_spmd(nc, in_maps, core_ids=list(range(NC)))

    out = np.empty((B, NQ, E), np.float32)
    for c in range(NC):
        b, r0 = c // 4, (c % 4) * NQC
        out[b, r0:r0 + NQC, :] = res.results[c]["out"]
    return out
